# revision 1
# baseline (speedup 1.0000x reference)
"""Trainium2 Bass kernel for nn_BuildK (27-neighborhood kNN softmax weights).

Strategy: shard the y dimension across 8 NeuronCores (spatial parallel, no
cross-core communication). Each core receives a halo-extended, x-rotated input
slab. Per voxel:

- Sort keys are u32 bit-packs: (bits(|diff|) & ~0x3F) | (2*w | signbit) built
  with DVE bitwise ops; a pruned top-8-of-26 selection network runs f32
  min/max on the packed keys (positive-float order == bit order, and min/max
  select operands bit-exactly, so the 6-bit payload survives).
- Unpack recovers the signed centered neighbor value Vk_r = v_q - c (pure bit
  ops) and the offset code 2*w per rank.
- Centered feature rows (rank 0 is identically 0) make f16 dot products safe:
  9*n2_d = diff9*(diff9 + 2*Sb - 2*Sa) + 9*(SSb + SSa) - 18*dot_ab, assembled
  per offset as pure tensor-tensor ops on the otherwise-idle Pool engine from
  pre-scaled planes (X9 = 9*x on ScalarE, P2 = (2*S, 9*SS) with x-rotated
  copies), plus an 8-wide f16 dot (product + strided pairwise-add tree in DVE
  2x mode) and a single fused STT per offset on DVE.
- logits = -n2/(2 sigma^2 ks^2) via a precomputed -1/(9*2 var ks^2) plane
  (softmax makes the reference's +1e-6 eps terms negligible; they are
  dropped). exp on ScalarE; the center's est is identically 1.0.
- Rank routing: per offset, a u16 is_equal mask (single-src 4x-mode TS)
  against the rank-stacked code tile, then copy_predicated writes the est
  plane (broadcast over the 8-rank axis) into the per-rank numerators — the
  masks are disjoint and exhaustive, so no accumulation or init is needed.

Output is gathered and reassembled (transposed) on the host.
"""

import sys

sys.path.insert(0, "/opt/trn_rl_repo")

import numpy as np

H, M, N = 64, 128, 128
NCORES = 8
YS = M // NCORES          # 16 owned y rows per core
YE = YS + 2               # 18 = sort region (owned + 1 halo each side)
YI = YS + 4               # 20 = input slab y extent (halo 2)
ZE = H + 2                # 66 = z extent with periodic wrap rows
KN = 9
ZC = 32                   # z chunk (2 chunks)
FS = ZC * YE              # 576 free elems in sort-phase ops
DS = ZC * YS              # 512 free elems in dots-phase ops

MASK_HI = 0x7FFFFFC0      # clears sign bit + 6 payload bits
MASK_ID = 0x3E            # payload: offset code 2*w (bits 1..5)

SELECT_MODE = "cpred"     # "stt" (stt + add chain) or "cpred" (TS mask + copy_predicated)
ADD_POOL_EVERY = 1000     # 1-in-n rank-acc adds on Pool
MULT_POOL_EVERY = 1000    # 1-in-n 8-wide dot multiplies on Pool
TREE_POOL_EVERY = 1000    # 1-in-n pt4 tree levels on Pool
ASM_DVE = ()              # which of the 7 assembly TT sites run on DVE


# --------------------------------------------------------------------------
# Selection network: top-8-sorted of the 26 non-center candidates (center is
# always rank 0).  Identical topology to the baseline.
# --------------------------------------------------------------------------

_SORT9 = [(0, 3), (1, 7), (2, 5), (4, 8), (0, 7), (2, 4), (3, 8), (5, 6),
          (0, 2), (1, 3), (4, 5), (7, 8), (1, 4), (3, 6), (5, 7), (0, 1),
          (2, 4), (3, 5), (6, 8), (2, 3), (4, 5), (6, 7), (1, 2), (3, 4),
          (5, 6)]


def _oddeven_merge(lo, n, r, out):
    step = r * 2
    if step < n:
        _oddeven_merge(lo, n, step, out)
        _oddeven_merge(lo + r, n, step, out)
        for i in range(lo + r, lo + n - r, step):
            out.append((i, i + r))
    else:
        out.append((lo, lo + r))


def _merge_topk(lenA, lenB, k):
    ces = []
    _oddeven_merge(0, 32, 1, ces)
    inf = [False] * 32
    for w in range(lenA, 16):
        inf[w] = True
    for w in range(16 + lenB, 32):
        inf[w] = True
    label = list(range(32))
    kept = []
    for (i, j) in ces:
        if inf[i] and inf[j]:
            continue
        if inf[j] and not inf[i]:
            continue
        if inf[i] and not inf[j]:
            label[i], label[j] = label[j], label[i]
            inf[i], inf[j] = False, True
            continue
        kept.append((label[i], label[j]))
    needed = set(label[w] for w in range(k))
    keep = []
    for (i, j) in reversed(kept):
        if i in needed or j in needed:
            keep.append((i, j))
            needed.add(i)
            needed.add(j)
    keep.reverse()

    def rm(w):
        return w if w < 16 else w - 16 + lenA

    return [(rm(i), rm(j)) for (i, j) in keep], [rm(label[w]) for w in range(k)]


def build_network():
    cand = [d for d in range(27) if d != 13]
    S8 = [(0, 1), (2, 3), (4, 5), (6, 7), (0, 2), (1, 3), (4, 6), (5, 7),
          (1, 2), (5, 6), (0, 4), (3, 7), (1, 5), (2, 6), (1, 4), (3, 6),
          (2, 4), (3, 5), (3, 4)]
    net = []
    net += [(i, j) for (i, j) in _SORT9]
    net += [(i + 9, j + 9) for (i, j) in _SORT9]
    net += [(i + 18, j + 18) for (i, j) in S8]
    m1, ow1 = _merge_topk(9, 9, 8)
    net += m1
    m2, ow2 = _merge_topk(8, 8, 8)
    remap = {i: ow1[i] for i in range(8)}
    remap.update({8 + i: 18 + i for i in range(8)})
    net += [(remap[i], remap[j]) for (i, j) in m2]
    outw = [remap[w] for w in ow2]

    live = set(outw)
    ops = []
    for (i, j) in reversed(net):
        ni, nj = i in live, j in live
        if not (ni or nj):
            continue
        ops.append((i, j, ni, nj))
        live.add(i)
        live.add(j)
    ops.reverse()
    return ops, outw, cand


NET_OPS, NET_OUTW, CAND = build_network()

OFFS = [(oz, oy, ox) for oz in (-1, 0, 1) for oy in (-1, 0, 1)
        for ox in (-1, 0, 1)]            # reference enumeration; 13 = center

# code of candidate w (payload bits 1..5); CAND[w] = offset index d
CODE_OF_D = {d: 2 * w for w, d in enumerate(CAND)}
DI_OF_D = {d: i for i, d in enumerate(CAND)}


def _slot_peak():
    """Peak concurrent kbig slots used by prep + network."""
    used = 26
    peak = 26
    live = 26
    for (i, j, ni, nj) in NET_OPS:
        live += (1 if ni else 0) + (1 if nj else 0)
        peak = max(peak, live)
        live -= 2
        live += (1 if ni else 0) + (1 if nj else 0)
        # correction: the two inputs free only after both outputs written
    return peak


NSLOT = 30


# --------------------------------------------------------------------------
# Bass graph
# --------------------------------------------------------------------------

def build_bass(ks_value: float, reps: int = 1, markers=None):
    from concourse import bacc, mybir
    from concourse import tile
    from concourse.alu_op_type import AluOpType as op

    f32 = mybir.dt.float32
    f16 = mybir.dt.float16
    u32 = mybir.dt.uint32
    u16 = mybir.dt.uint16
    AF = mybir.ActivationFunctionType

    nc = bacc.Bacc("TRN2", target_bir_lowering=False, debug=False,
                   num_devices=NCORES)

    def mark(label):
        if markers is not None:
            markers.append((label, nc.next_id()))

    xin = nc.dram_tensor("xin", [128, 3, ZE, YI], f32, kind="ExternalInput").ap()
    outd = nc.dram_tensor("out", [128, KN, H, YS], f32,
                          kind="ExternalOutput").ap()

    dve = nc.vector
    act = nc.scalar
    gp = nc.gpsimd

    with tile.TileContext(nc) as tc:
      for _rep in range(reps):
        with tc.tile_pool(name="pp", bufs=1) as pp:
            X3 = pp.tile([128, 3, ZE, YI], f32, tag="X3")
            nc.sync.dma_start(out=X3[:], in_=xin[:])
            Vk = pp.tile([128, 8, ZE, YE], f16, tag="Vk")
            icod = pp.tile([128, 8, H, YS], f16, tag="icod")
            maskC = pp.tile([128, 1], u32, tag="maskC")
            dve.memset(maskC[:], MASK_HI)
            P2 = pp.tile([128, 3, 2, ZE, YE], f32, tag="P2")
            negsc = pp.tile([128, H, YS], f32, tag="negsc")
            SaM2 = pp.tile([128, H, YS], f32, tag="SaM2")

            # ---------------- sort phase ----------------
            with tc.tile_pool(name="sortp", bufs=1) as sp:
                kbig = sp.tile([128, NSLOT, FS], f32, tag="kbig")
                VkT = sp.tile([128, 8, ZC, YE], f32, tag="VkT")
                dtmp = [sp.tile([128, FS], f32, name=f"dt{i}", tag=f"dt{i}")
                        for i in range(3)]
                s2tmp = [sp.tile([128, FS], u32, name=f"st{i}", tag=f"st{i}")
                         for i in range(3)]
                ictmp = [sp.tile([128, FS], u32, name=f"ic{i}", tag=f"ic{i}")
                         for i in range(2)]

                for zc in range(0, H, ZC):
                    mark("sort_chunk")
                    cvw = X3[:, 1, 1 + zc:1 + zc + ZC, 1:1 + YE]

                    def vview(d):
                        oz, oy, ox = OFFS[d]
                        return X3[:, ox + 1,
                                  1 + zc + oz:1 + zc + oz + ZC,
                                  1 + oy:1 + oy + YE]

                    free_slots = list(range(NSLOT))
                    wire_slot = {}

                    def k_ap(s):
                        return kbig[:, s, :]

                    for w, d in enumerate(CAND):
                        s = free_slots.pop()
                        wire_slot[w] = s
                        dt = dtmp[w % 3]
                        gp.tensor_tensor(out=dt[:], in0=vview(d), in1=cvw,
                                         op=op.subtract)
                        du = dt[:].bitcast(u32)
                        st = s2tmp[w % 3]
                        dve.tensor_scalar(out=st[:], in0=du, scalar1=31,
                                          scalar2=2 * w,
                                          op0=op.logical_shift_right,
                                          op1=op.bitwise_or)
                        dve.scalar_tensor_tensor(out=k_ap(s).bitcast(u32),
                                                 in0=du, scalar=maskC[:],
                                                 in1=st[:],
                                                 op0=op.bitwise_and,
                                                 op1=op.bitwise_or)

                    for (i, j, ni, nj) in NET_OPS:
                        si, sj = wire_slot[i], wire_slot[j]
                        new_i = free_slots.pop() if ni else None
                        new_j = free_slots.pop() if nj else None
                        if ni:
                            dve.tensor_tensor(out=k_ap(new_i), in0=k_ap(si),
                                              in1=k_ap(sj), op=op.min)
                        if nj:
                            dve.tensor_tensor(out=k_ap(new_j), in0=k_ap(si),
                                              in1=k_ap(sj), op=op.max)
                        free_slots.append(si)
                        free_slots.append(sj)
                        if ni:
                            wire_slot[i] = new_i
                        else:
                            del wire_slot[i]
                        if nj:
                            wire_slot[j] = new_j
                        else:
                            del wire_slot[j]

                    # unpack ranks 1..8: Vk (signed centered value) + icod
                    for r in range(1, KN):
                        key = k_ap(wire_slot[NET_OUTW[r - 1]]).bitcast(u32)
                        st = s2tmp[r % 3]
                        dve.tensor_scalar(out=st[:], in0=key, scalar1=31,
                                          scalar2=None,
                                          op0=op.logical_shift_left)
                        dve.scalar_tensor_tensor(
                            out=VkT[:, r - 1].bitcast(u32), in0=key,
                            scalar=maskC[:], in1=st[:], op0=op.bitwise_and,
                            op1=op.bitwise_or)
                        ic = ictmp[r % 2]
                        dve.tensor_scalar(out=ic[:], in0=key,
                                          scalar1=MASK_ID, scalar2=None,
                                          op0=op.bitwise_and)
                        icv = ic[:].rearrange("p (z y) -> p z y", z=ZC, y=YE)
                        gp.tensor_copy(out=icod[:, r - 1, zc:zc + ZC, :],
                                       in_=icv[:, :, 1:1 + YS])

                    act.activation(out=Vk[:, :, 1 + zc:1 + zc + ZC, :],
                                   in_=VkT[:], func=AF.Copy)

            # z wrap rows of Vk
            nc.sync.dma_start(out=Vk[:, :, 0:1, :], in_=Vk[:, :, H:H + 1, :])
            nc.sync.dma_start(out=Vk[:, :, ZE - 1:ZE, :], in_=Vk[:, :, 1:2, :])

            # ---------------- stats: Sa, SSa, var, scale ----------------
            mark("stats")
            with tc.tile_pool(name="statp", bufs=1) as stp:
                sq = stp.tile([128, 8, ZE, YE], f16, tag="sq")
                t4 = stp.tile([128, 4, ZE, YE], f16, tag="t4")
                t2 = stp.tile([128, 2, ZE, YE], f16, tag="t2")
                tS = stp.tile([128, ZE, YE], f32, tag="tS")
                tSS = stp.tile([128, ZE, YE], f32, tag="tSS")
                v1 = stp.tile([128, H, YS], f32, tag="v1")
                v2 = stp.tile([128, H, YS], f32, tag="v2")

                dve.tensor_tensor(out=t4[:], in0=Vk[:, 0:4], in1=Vk[:, 4:8],
                                  op=op.add)
                dve.tensor_tensor(out=t2[:], in0=t4[:, 0:2], in1=t4[:, 2:4],
                                  op=op.add)
                dve.tensor_tensor(out=tS[:], in0=t2[:, 0], in1=t2[:, 1],
                                  op=op.add)
                act.activation(out=sq[:], in_=Vk[:], func=AF.Square)
                dve.tensor_tensor(out=t4[:], in0=sq[:, 0:4], in1=sq[:, 4:8],
                                  op=op.add)
                dve.tensor_tensor(out=t2[:], in0=t4[:, 0:2], in1=t4[:, 2:4],
                                  op=op.add)
                dve.tensor_tensor(out=tSS[:], in0=t2[:, 0], in1=t2[:, 1],
                                  op=op.add)
                # P2 stores (2*S, 9*SS) so the logit assembly is pure TT
                dve.tensor_scalar(out=P2[:, 1, 0], in0=tS[:], scalar1=2.0,
                                  scalar2=None, op0=op.mult)
                gp.tensor_scalar(out=P2[:, 1, 1], in0=tSS[:], scalar1=9.0,
                                 scalar2=None, op0=op.mult)

                # x rotations of the (Sa, SSa) planes
                nc.sync.dma_start(out=P2[:, 0][1:128], in_=P2[:, 1][0:127])
                nc.sync.dma_start(out=P2[:, 0][0:1], in_=P2[:, 1][127:128])
                nc.sync.dma_start(out=P2[:, 2][0:127], in_=P2[:, 1][1:128])
                nc.sync.dma_start(out=P2[:, 2][127:128], in_=P2[:, 1][0:1])

                SaO = tS[:, 1:1 + H, 1:1 + YS]
                SSaO = tSS[:, 1:1 + H, 1:1 + YS]
                dve.tensor_scalar(out=SaM2[:], in0=SaO, scalar1=-2.0,
                                  scalar2=None, op0=op.mult)
                # var8 = 8*sigma^2 = SSa - Sa^2/9
                dve.scalar_tensor_tensor(out=v1[:], in0=SaO,
                                         scalar=-1.0 / 9.0, in1=SaO,
                                         op0=op.mult, op1=op.mult)
                gp.tensor_tensor(out=v1[:], in0=v1[:], in1=SSaO, op=op.add)
                # zero guard + negsc = -4/(ks^2 * var8), 0 where var8 == 0
                dve.tensor_scalar(out=v2[:], in0=v1[:], scalar1=0.0,
                                  scalar2=None, op0=op.is_equal)
                dve.tensor_tensor(out=v2[:], in0=v2[:], in1=v1[:], op=op.add)
                dve.reciprocal(out=v2[:], in_=v2[:])
                dve.tensor_scalar(out=v1[:], in0=v1[:], scalar1=0.0,
                                  scalar2=None, op0=op.not_equal)
                dve.tensor_tensor(out=v1[:], in0=v1[:], in1=v2[:], op=op.mult)
                # 1/9 absorbs the x9 scaling of the assembly terms
                dve.tensor_scalar(out=negsc[:], in0=v1[:],
                                  scalar1=-4.0 / (9.0 * ks_value * ks_value),
                                  scalar2=None, op0=op.mult)

            # ---------------- dots + select + softmax ----------------
            with tc.tile_pool(name="dotp", bufs=1) as dp:
                for zc in range(0, H, ZC):
                    mark("dots_chunk")
                    wrm = dp.tile([128, 8, ZC + 2, YE], f16, tag="wrm")
                    wrp = dp.tile([128, 8, ZC + 2, YE], f16, tag="wrp")
                    src = Vk[:, :, zc:zc + ZC + 2, :]
                    nc.sync.dma_start(out=wrm[1:128], in_=src[0:127])
                    nc.sync.dma_start(out=wrm[0:1], in_=src[127:128])
                    nc.sync.dma_start(out=wrp[0:127], in_=src[1:128])
                    nc.sync.dma_start(out=wrp[127:128], in_=src[0:1])

                    X9c = dp.tile([128, 3, ZC + 2, YI], f32, tag="X9c")
                    act.activation(out=X9c[:],
                                   in_=X3[:, :, zc:zc + ZC + 2, :],
                                   func=AF.Copy, scale=9.0)
                    prods = [dp.tile([128, 8, ZC, YS], f16, name=f"pr{i}",
                                     tag=f"pr{i}") for i in range(2)]
                    pt4s = [dp.tile([128, 4, ZC, YS], f16, name=f"p4{i}",
                                    tag=f"p4{i}") for i in range(2)]
                    pt2s = [dp.tile([128, 2, ZC, YS], f16, name=f"p2{i}",
                                    tag=f"p2{i}") for i in range(2)]
                    dotps = [dp.tile([128, ZC, YS], f16, name=f"dp{i}",
                                     tag=f"dp{i}") for i in range(2)]
                    acc = dp.tile([128, 8, ZC, YS], f16, tag="acc")
                    if SELECT_MODE == "rank":
                        est26 = dp.tile([128, 26, ZC, YS], f16, tag="est26")
                    elif SELECT_MODE in ("stt", "cpred2"):
                        acc2 = dp.tile([128, 8, ZC, YS], f16, tag="acc2")
                    if SELECT_MODE == "cpred2":
                        gp.memset(acc[:], 0.0)
                        gp.memset(acc2[:], 0.0)
                    if SELECT_MODE != "rank":
                        mdt = f16 if SELECT_MODE == "stt" else u16
                        msk = [dp.tile([128, 8, ZC, YS], mdt, name=f"mk{i}",
                                       tag=f"mk{i}") for i in range(2)]
                    if SELECT_MODE != "rank":
                        estt = [dp.tile([128, ZC, YS], f16, name=f"es{i}",
                                        tag=f"es{i}") for i in range(2)]
                    nmt = 4 if SELECT_MODE == "rank" else 6
                    mt = [dp.tile([128, ZC, YS], f32, name=f"mt{i}",
                          tag=f"mt{i}") for i in range(nmt)]

                    VkC = Vk[:, :, 1 + zc:1 + zc + ZC, 1:1 + YS]
                    scv = negsc[:, zc:zc + ZC, :]
                    sav = SaM2[:, zc:zc + ZC, :]
                    icv8 = icod[:, :, zc:zc + ZC, :]

                    for d in range(27):
                        if d == 13:
                            continue
                        oz, oy, ox = OFFS[d]
                        if ox == 0:
                            vb = Vk[:, :, 1 + zc + oz:1 + zc + oz + ZC,
                                    1 + oy:1 + oy + YS]
                        else:
                            wrt = wrm if ox == -1 else wrp
                            vb = wrt[:, :, 1 + oz:1 + oz + ZC,
                                     1 + oy:1 + oy + YS]
                        Sbv = P2[:, ox + 1, 0,
                                 1 + zc + oz:1 + zc + oz + ZC,
                                 1 + oy:1 + oy + YS]
                        SSbv = P2[:, ox + 1, 1,
                                  1 + zc + oz:1 + zc + oz + ZC,
                                  1 + oy:1 + oy + YS]
                        w4 = [mt[(4 * d + j) % nmt] for j in range(4)]

                        def aeng_(site):
                            return dve if site in ASM_DVE else gp

                        dfv = w4[0]
                        aeng_(0).tensor_tensor(
                            out=dfv[:],
                            in0=X9c[:, ox + 1, 1 + oz:1 + oz + ZC,
                                    2 + oy:2 + oy + YS],
                            in1=X9c[:, 1, 1:1 + ZC, 2:2 + YS],
                            op=op.subtract)
                        m1 = w4[1]
                        aeng_(1).tensor_tensor(out=m1[:], in0=Sbv, in1=sav,
                                               op=op.add)
                        m2 = w4[2]
                        aeng_(2).tensor_tensor(out=m2[:], in0=dfv[:],
                                               in1=m1[:], op=op.add)
                        m3 = w4[1]
                        aeng_(3).tensor_tensor(out=m3[:], in0=dfv[:],
                                               in1=m2[:], op=op.mult)
                        # 8-wide centered dot
                        prod = prods[d % 2]
                        pt4 = pt4s[d % 2]
                        pt2 = pt2s[d % 2]
                        dotp = dotps[d % 2]
                        peng = gp if d % MULT_POOL_EVERY == 2 else dve
                        peng.tensor_tensor(out=prod[:], in0=VkC, in1=vb,
                                           op=op.mult)
                        teng = gp if d % TREE_POOL_EVERY == 1 else dve
                        teng.tensor_tensor(out=pt4[:], in0=prod[:, 0:4],
                                           in1=prod[:, 4:8], op=op.add)
                        t2e = gp if d % 2 == 0 else dve
                        t2e.tensor_tensor(out=pt2[:], in0=pt4[:, 0:2],
                                          in1=pt4[:, 2:4], op=op.add)
                        tde = gp if d % 2 == 1 else dve
                        tde.tensor_tensor(out=dotp[:], in0=pt2[:, 0],
                                          in1=pt2[:, 1], op=op.add)
                        # 9*(SSb + SSa)
                        m4 = w4[3]
                        aeng_(4).tensor_tensor(out=m4[:], in0=SSbv,
                                               in1=P2[:, 1, 1,
                                                      1 + zc:1 + zc + ZC,
                                                      1:1 + YS], op=op.add)
                        m5 = w4[0]
                        dve.scalar_tensor_tensor(out=m5[:], in0=dotp[:],
                                                 scalar=-18.0, in1=m4[:],
                                                 op0=op.mult, op1=op.add)
                        aeng_(5).tensor_tensor(out=m5[:], in0=m5[:],
                                               in1=m3[:], op=op.add)
                        aeng_(6).tensor_tensor(out=m4[:], in0=m5[:], in1=scv,
                                               op=op.mult)
                        if SELECT_MODE == "rank":
                            ee = est26[:, DI_OF_D[d]]
                        else:
                            ee = estt[d % 2][:]
                        act.activation(out=ee, in_=m4[:], func=AF.Exp)
                        code = float(CODE_OF_D[d])
                        if SELECT_MODE == "rank":
                            pass  # routing happens after the d loop
                        elif SELECT_MODE in ("cpred", "cpred2"):
                            eb = ee.rearrange("p (a z) y -> p a z y", a=1)
                            ebb = eb.broadcast_to([128, 8, ZC, YS])
                            mm = msk[d % 2]
                            dve.tensor_scalar(out=mm[:], in0=icv8,
                                              scalar1=code, scalar2=None,
                                              op0=op.is_equal)
                            at = (acc if (SELECT_MODE == "cpred"
                                          or d % 2 == 0) else acc2)
                            dve.copy_predicated(out=at[:], mask=mm[:],
                                                data=ebb)
                        else:
                            eb = ee.rearrange("p (a z) y -> p a z y", a=1)
                            ebb = eb.broadcast_to([128, 8, ZC, YS])
                            atile = acc if d % 2 == 0 else acc2
                            if d in (0, 1):
                                dve.scalar_tensor_tensor(
                                    out=atile[:], in0=icv8, scalar=code,
                                    in1=ebb, op0=op.is_equal, op1=op.mult)
                            else:
                                mm = msk[d % 2]
                                dve.scalar_tensor_tensor(
                                    out=mm[:], in0=icv8, scalar=code,
                                    in1=ebb, op0=op.is_equal, op1=op.mult)
                                aeng = gp if d % ADD_POOL_EVERY == 1 else dve
                                aeng.tensor_tensor(out=atile[:],
                                                   in0=atile[:],
                                                   in1=mm[:], op=op.add)

                    if SELECT_MODE == "rank":
                        ocr = [dp.tile([128, ZC, YS], f16, name=f"oc{i}",
                                       tag=f"oc{i}") for i in range(4)]
                        for r in range(8):
                            icr = icod[:, r, zc:zc + ZC, :]
                            av = acc[:, r]
                            for j, d in enumerate(
                                    dd for dd in range(27) if dd != 13):
                                code = float(CODE_OF_D[d])
                                ev = est26[:, j]
                                if j == 0:
                                    dve.scalar_tensor_tensor(
                                        out=av, in0=icr, scalar=code,
                                        in1=ev, op0=op.is_equal,
                                        op1=op.mult)
                                else:
                                    oc = ocr[(r + j) % 4]
                                    dve.scalar_tensor_tensor(
                                        out=oc[:], in0=icr, scalar=code,
                                        in1=ev, op0=op.is_equal,
                                        op1=op.mult)
                                    aeng = (gp if (r * 26 + j)
                                            % ADD_POOL_EVERY == 1 else dve)
                                    aeng.tensor_tensor(out=av, in0=av,
                                                       in1=oc[:], op=op.add)

                    mark("softmax")
                    if SELECT_MODE in ("stt", "cpred2"):
                        dve.tensor_tensor(out=acc[:], in0=acc[:],
                                          in1=acc2[:], op=op.add)
                    pt4 = pt4s[0]
                    pt2 = pt2s[0]
                    dotp = dotps[0]
                    dve.tensor_tensor(out=pt4[:], in0=acc[:, 0:4],
                                      in1=acc[:, 4:8], op=op.add)
                    dve.tensor_tensor(out=pt2[:], in0=pt4[:, 0:2],
                                      in1=pt4[:, 2:4], op=op.add)
                    dve.tensor_tensor(out=dotp[:], in0=pt2[:, 0],
                                      in1=pt2[:, 1], op=op.add)
                    den = mt[0]
                    # +1.0 is the center's est (exp(0)); also correct for the
                    # sigma==0 uniform case where every est is 1.
                    dve.tensor_scalar(out=den[:], in0=dotp[:], scalar1=1.0,
                                      scalar2=None, op0=op.add)
                    rec = mt[1]
                    dve.reciprocal(out=rec[:], in_=den[:])
                    ob = dp.tile([128, KN, ZC, YS], f32, tag="ob")
                    act.activation(out=ob[:, 0], in_=rec[:], func=AF.Copy)
                    for r in range(1, KN):
                        eng = gp if r % 3 == 0 else dve
                        eng.tensor_tensor(out=ob[:, r], in0=acc[:, r - 1],
                                          in1=rec[:], op=op.mult)
                    nc.sync.dma_start(out=outd[:, :, zc:zc + ZC, :],
                                      in_=ob[:])

    mark("end")
    nc.compile()
    return nc


# --------------------------------------------------------------------------
# Host side
# --------------------------------------------------------------------------

_CACHED = {}


def _get_nc(ks_value):
    key = float(ks_value)
    if key not in _CACHED:
        _CACHED[key] = build_bass(key)
    return _CACHED[key]


def _shard_inputs(x):
    """x: [H, M, N] f32 -> list of per-core xin arrays [128, 3, ZE, YI]."""
    maps = []
    zext = np.arange(-1, H + 1) % H
    xs = np.arange(N)
    for c in range(NCORES):
        ys = (np.arange(YS * c - 2, YS * c + YS + 2)) % M
        slab = x[zext][:, ys, :]                       # [66, 20, 128]
        a = np.empty((128, 3, ZE, YI), dtype=np.float32)
        for r in range(3):
            xrot = (xs + r - 1) % N
            a[:, r] = slab[:, :, xrot].transpose(2, 0, 1)
        maps.append({"xin": np.ascontiguousarray(a)})
    return maps


def kernel(input, ksigma, k, w):
    from concourse.bass_utils import run_bass_kernel_spmd

    x = np.asarray(input, dtype=np.float32)
    assert x.shape == (H, M, N)
    ks = float(np.asarray(ksigma).reshape(-1)[0])
    assert int(k) == KN and int(w) == 3

    nc = _get_nc(ks)
    in_maps = _shard_inputs(x)
    res = run_bass_kernel_spmd(nc, in_maps, core_ids=list(range(NCORES)))
    full = np.empty((H, M, N, KN), dtype=np.float32)
    for c in range(NCORES):
        oc = res.results[c]["out"]          # [128, KN, H, YS]
        full[:, YS * c:YS * c + YS] = oc.transpose(2, 3, 0, 1)
    return full.reshape(H * M * N, KN)



# revision 7
# speedup vs baseline: 1.1734x; 1.1734x over previous
"""Trainium2 Bass kernel for nn_BuildK (27-neighborhood kNN softmax weights).

Strategy: shard the y dimension across 8 NeuronCores (spatial parallel, no
cross-core communication). Each core receives a halo-extended input slab;
the two x-rotated frames are built on-device with partition-shift DMAs.

- Sort keys are u32 bit-packs: (bits(|diff|) & ~0x3F) | (2*w | signbit) built
  with DVE bitwise ops; a pruned top-8-of-26 selection network runs f32
  min/max on the packed keys (positive-float order == bit order, and min/max
  select operands bit-exactly, so the 6-bit payload survives). Network ops
  are split between DVE and Pool (Pool min/max runs at default gpsimd
  efficiency, close to DVE speed, and is otherwise idle in this phase).
- Unpack recovers the signed centered neighbor value Vk_r = v_q - c (pure bit
  ops) and the offset code 2*w per rank.
- Pairwise distances exploit symmetry: 9*n2_d(p) = 9*||W(p)-W(p+d)||^2 is
  computed for only the 13 positive offsets d, over a (+1 z/y halo) extended
  domain; the -d direction reads the same field at view (p-d), with a
  partition-rotate DMA supplying the x-shifted copy when ox != 0. The field
  assembly is exact: 9*n2 = dfv*(dfv + 2Sb - 2Sa) + 9*(SSb+SSa) - 18*dot,
  with an 8-wide centered f16 dot (rank 0 is identically 0).
- logits = n2 * negsc with negsc = -1/(9*2 sigma^2 ks^2) per voxel (the
  reference's +1e-6 eps is negligible under softmax and dropped). exp on
  ScalarE; the center's est is identically 1.0.
- Rank routing: per direction, a u16 is_equal mask (4x-mode TS) against the
  rank-stacked code tile, then copy_predicated writes the est plane
  (broadcast over the 8-rank axis) into the per-rank numerators - the masks
  are disjoint and exhaustive, so no accumulation or init is needed.
- Output is written f16 (softmax weights in [0,1]; quantization ~5e-4 rel)
  and upcast on the host.
"""

import sys

sys.path.insert(0, "/opt/trn_rl_repo")

import numpy as np

H, M, N = 64, 128, 128
NCORES = 8
YS = M // NCORES          # 16 owned y rows per core
YE = YS + 2               # 18 = sort region (owned + 1 halo each side)
YI = YS + 4               # 20 = input slab y extent (halo 2)
ZE = H + 2                # 66 = z extent with periodic wrap rows
KN = 9
ZC = 32                   # z chunk (2 chunks)
FS = ZC * YE              # 576 free elems in sort-phase ops

MASK_HI = 0x7FFFFFC0      # clears sign bit + 6 payload bits
MASK_ID = 0x3E            # payload: offset code 2*w (bits 1..5)

# ---- engine-assignment knobs (tuned against TimelineSim) ----
# Real-ISA constraints (probed on neuronxcc): Pool supports TT add/sub/mult,
# TS mult/is_equal, copies; NOT min/max, NOT bitwise/shift, NOT STT, NOT
# copy_predicated. The sort network and routing are therefore DVE-locked.
NET_POOL_A, NET_POOL_B = 0, 5   # network op i -> Pool iff (i % B) < A (0: ISA-illegal)
PREP_TS_POOL = False            # shift/or pack op (bitwise: DVE only)
UNPACK_TS_POOL = False          # shift-left of unpack (bitwise: DVE only)
ASM_DVE_SITES = ()              # which of the 6 assembly TT sites run on DVE
TREE_POOL_LVL = (1, 2)          # which dot-tree levels (0=pt4,1=pt2,2=dotp) on Pool
MULT_POOL = True                # lg = n2 * scv on Pool


# --------------------------------------------------------------------------
# Selection network: top-8-sorted of the 26 non-center candidates (center is
# always rank 0).
# --------------------------------------------------------------------------

_SORT9 = [(0, 3), (1, 7), (2, 5), (4, 8), (0, 7), (2, 4), (3, 8), (5, 6),
          (0, 2), (1, 3), (4, 5), (7, 8), (1, 4), (3, 6), (5, 7), (0, 1),
          (2, 4), (3, 5), (6, 8), (2, 3), (4, 5), (6, 7), (1, 2), (3, 4),
          (5, 6)]


def _oddeven_merge(lo, n, r, out):
    step = r * 2
    if step < n:
        _oddeven_merge(lo, n, step, out)
        _oddeven_merge(lo + r, n, step, out)
        for i in range(lo + r, lo + n - r, step):
            out.append((i, i + r))
    else:
        out.append((lo, lo + r))


def _merge_topk(lenA, lenB, k):
    ces = []
    _oddeven_merge(0, 32, 1, ces)
    inf = [False] * 32
    for w in range(lenA, 16):
        inf[w] = True
    for w in range(16 + lenB, 32):
        inf[w] = True
    label = list(range(32))
    kept = []
    for (i, j) in ces:
        if inf[i] and inf[j]:
            continue
        if inf[j] and not inf[i]:
            continue
        if inf[i] and not inf[j]:
            label[i], label[j] = label[j], label[i]
            inf[i], inf[j] = False, True
            continue
        kept.append((label[i], label[j]))
    needed = set(label[w] for w in range(k))
    keep = []
    for (i, j) in reversed(kept):
        if i in needed or j in needed:
            keep.append((i, j))
            needed.add(i)
            needed.add(j)
    keep.reverse()

    def rm(w):
        return w if w < 16 else w - 16 + lenA

    return [(rm(i), rm(j)) for (i, j) in keep], [rm(label[w]) for w in range(k)]


def build_network():
    cand = [d for d in range(27) if d != 13]
    S8 = [(0, 1), (2, 3), (4, 5), (6, 7), (0, 2), (1, 3), (4, 6), (5, 7),
          (1, 2), (5, 6), (0, 4), (3, 7), (1, 5), (2, 6), (1, 4), (3, 6),
          (2, 4), (3, 5), (3, 4)]
    net = []
    net += [(i, j) for (i, j) in _SORT9]
    net += [(i + 9, j + 9) for (i, j) in _SORT9]
    net += [(i + 18, j + 18) for (i, j) in S8]
    m1, ow1 = _merge_topk(9, 9, 8)
    net += m1
    m2, ow2 = _merge_topk(8, 8, 8)
    remap = {i: ow1[i] for i in range(8)}
    remap.update({8 + i: 18 + i for i in range(8)})
    net += [(remap[i], remap[j]) for (i, j) in m2]
    outw = [remap[w] for w in ow2]

    live = set(outw)
    ops = []
    for (i, j) in reversed(net):
        ni, nj = i in live, j in live
        if not (ni or nj):
            continue
        ops.append((i, j, ni, nj))
        live.add(i)
        live.add(j)
    ops.reverse()
    return ops, outw, cand


NET_OPS, NET_OUTW, CAND = build_network()

OFFS = [(oz, oy, ox) for oz in (-1, 0, 1) for oy in (-1, 0, 1)
        for ox in (-1, 0, 1)]            # reference enumeration; 13 = center
POS13 = [d for d in range(13)]           # positive offsets: OFFS[0..12]

# code of candidate w (payload bits 1..5); CAND[w] = offset index d
CODE_OF_D = {d: 2 * w for w, d in enumerate(CAND)}

NSLOT = 30


# --------------------------------------------------------------------------
# Bass graph
# --------------------------------------------------------------------------

def build_bass(ks_value: float, reps: int = 1, markers=None):
    from concourse import bacc, mybir
    from concourse import tile
    from concourse.alu_op_type import AluOpType as op

    f32 = mybir.dt.float32
    f16 = mybir.dt.float16
    u32 = mybir.dt.uint32
    u16 = mybir.dt.uint16
    AF = mybir.ActivationFunctionType

    nc = bacc.Bacc("TRN2", target_bir_lowering=False, debug=False,
                   num_devices=NCORES)

    def mark(label):
        if markers is not None:
            markers.append((label, nc.next_id()))

    xin = nc.dram_tensor("xin", [128, ZE, YI], f32, kind="ExternalInput").ap()
    outd = nc.dram_tensor("out", [128, KN, H, YS], f16,
                          kind="ExternalOutput").ap()

    dve = nc.vector
    act = nc.scalar
    gp = nc.gpsimd

    with tile.TileContext(nc) as tc:
      for _rep in range(reps):
        with tc.tile_pool(name="pp", bufs=1) as pp:
            X3 = pp.tile([128, 3, ZE, YI], f32, tag="X3")
            nc.sync.dma_start(out=X3[:, 1], in_=xin[:])
            # on-device x-rotated frames (frame r holds x-col p+r-1 at
            # partition p)
            nc.sync.dma_start(out=X3[:, 0][1:128], in_=X3[:, 1][0:127])
            nc.sync.dma_start(out=X3[:, 0][0:1], in_=X3[:, 1][127:128])
            nc.sync.dma_start(out=X3[:, 2][0:127], in_=X3[:, 1][1:128])
            nc.sync.dma_start(out=X3[:, 2][127:128], in_=X3[:, 1][0:1])
            Vk = pp.tile([128, 8, ZE, YE], f16, tag="Vk")
            icod = pp.tile([128, 8, H, YS], f16, tag="icod")
            maskC = pp.tile([128, 1], u32, tag="maskC")
            dve.memset(maskC[:], MASK_HI)
            P2 = pp.tile([128, 3, 2, ZE, YE], f32, tag="P2")
            negsc = pp.tile([128, H, YS], f32, tag="negsc")

            # ---------------- sort phase ----------------
            with tc.tile_pool(name="sortp", bufs=1) as sp:
                kbig = sp.tile([128, NSLOT, FS], f32, tag="kbig")
                VkT = sp.tile([128, 8, ZC, YE], f32, tag="VkT")
                dtmp = [sp.tile([128, FS], f32, name=f"dt{i}", tag=f"dt{i}")
                        for i in range(3)]
                s2tmp = [sp.tile([128, FS], u32, name=f"st{i}", tag=f"st{i}")
                         for i in range(3)]
                ictmp = [sp.tile([128, FS], u32, name=f"ic{i}", tag=f"ic{i}")
                         for i in range(2)]

                for zc in range(0, H, ZC):
                    mark("sort_chunk")
                    cvw = X3[:, 1, 1 + zc:1 + zc + ZC, 1:1 + YE]

                    def vview(d):
                        oz, oy, ox = OFFS[d]
                        return X3[:, ox + 1,
                                  1 + zc + oz:1 + zc + oz + ZC,
                                  1 + oy:1 + oy + YE]

                    free_slots = list(range(NSLOT))
                    wire_slot = {}

                    def k_ap(s):
                        return kbig[:, s, :]

                    for w, d in enumerate(CAND):
                        s = free_slots.pop()
                        wire_slot[w] = s
                        dt = dtmp[w % 3]
                        gp.tensor_tensor(out=dt[:], in0=vview(d), in1=cvw,
                                         op=op.subtract)
                        du = dt[:].bitcast(u32)
                        st = s2tmp[w % 3]
                        tse = gp if PREP_TS_POOL else dve
                        tse.tensor_scalar(out=st[:], in0=du, scalar1=31,
                                          scalar2=2 * w,
                                          op0=op.logical_shift_right,
                                          op1=op.bitwise_or)
                        dve.scalar_tensor_tensor(out=k_ap(s).bitcast(u32),
                                                 in0=du, scalar=maskC[:],
                                                 in1=st[:],
                                                 op0=op.bitwise_and,
                                                 op1=op.bitwise_or)

                    for ni_op, (i, j, ni, nj) in enumerate(NET_OPS):
                        si, sj = wire_slot[i], wire_slot[j]
                        new_i = free_slots.pop() if ni else None
                        new_j = free_slots.pop() if nj else None
                        eng = gp if (ni_op % NET_POOL_B) < NET_POOL_A else dve
                        if ni:
                            eng.tensor_tensor(out=k_ap(new_i), in0=k_ap(si),
                                              in1=k_ap(sj), op=op.min)
                        if nj:
                            eng.tensor_tensor(out=k_ap(new_j), in0=k_ap(si),
                                              in1=k_ap(sj), op=op.max)
                        free_slots.append(si)
                        free_slots.append(sj)
                        if ni:
                            wire_slot[i] = new_i
                        else:
                            del wire_slot[i]
                        if nj:
                            wire_slot[j] = new_j
                        else:
                            del wire_slot[j]

                    # unpack ranks 1..8: Vk (signed centered value) + icod
                    for r in range(1, KN):
                        key = k_ap(wire_slot[NET_OUTW[r - 1]]).bitcast(u32)
                        st = s2tmp[r % 3]
                        tse = gp if UNPACK_TS_POOL else dve
                        tse.tensor_scalar(out=st[:], in0=key, scalar1=31,
                                          scalar2=None,
                                          op0=op.logical_shift_left)
                        dve.scalar_tensor_tensor(
                            out=VkT[:, r - 1].bitcast(u32), in0=key,
                            scalar=maskC[:], in1=st[:], op0=op.bitwise_and,
                            op1=op.bitwise_or)
                        ic = ictmp[r % 2]
                        dve.tensor_scalar(out=ic[:], in0=key,
                                          scalar1=MASK_ID, scalar2=None,
                                          op0=op.bitwise_and)
                        icv = ic[:].rearrange("p (z y) -> p z y", z=ZC, y=YE)
                        gp.tensor_copy(out=icod[:, r - 1, zc:zc + ZC, :],
                                       in_=icv[:, :, 1:1 + YS])

                    act.activation(out=Vk[:, :, 1 + zc:1 + zc + ZC, :],
                                   in_=VkT[:], func=AF.Copy)

            # z wrap rows of Vk
            nc.sync.dma_start(out=Vk[:, :, 0:1, :], in_=Vk[:, :, H:H + 1, :])
            nc.sync.dma_start(out=Vk[:, :, ZE - 1:ZE, :], in_=Vk[:, :, 1:2, :])

            # X9 = 9*x, in place over X3 (sort no longer needs raw X3)
            act.activation(out=X3[:], in_=X3[:], func=AF.Copy, scale=9.0)

            # ---------------- stats: Sa, SSa, var, scale ----------------
            mark("stats")
            with tc.tile_pool(name="statp", bufs=1) as stp:
                sq = stp.tile([128, 8, ZE, YE], f16, tag="sq")
                t4 = stp.tile([128, 4, ZE, YE], f16, tag="t4")
                t2 = stp.tile([128, 2, ZE, YE], f16, tag="t2")
                tS = stp.tile([128, ZE, YE], f32, tag="tS")
                tSS = stp.tile([128, ZE, YE], f32, tag="tSS")
                v1 = stp.tile([128, H, YS], f32, tag="v1")
                v2 = stp.tile([128, H, YS], f32, tag="v2")

                dve.tensor_tensor(out=t4[:], in0=Vk[:, 0:4], in1=Vk[:, 4:8],
                                  op=op.add)
                dve.tensor_tensor(out=t2[:], in0=t4[:, 0:2], in1=t4[:, 2:4],
                                  op=op.add)
                dve.tensor_tensor(out=tS[:], in0=t2[:, 0], in1=t2[:, 1],
                                  op=op.add)
                act.activation(out=sq[:], in_=Vk[:], func=AF.Square)
                dve.tensor_tensor(out=t4[:], in0=sq[:, 0:4], in1=sq[:, 4:8],
                                  op=op.add)
                dve.tensor_tensor(out=t2[:], in0=t4[:, 0:2], in1=t4[:, 2:4],
                                  op=op.add)
                dve.tensor_tensor(out=tSS[:], in0=t2[:, 0], in1=t2[:, 1],
                                  op=op.add)
                # P2 stores (2*S, 9*SS) so the n2 assembly is pure TT
                dve.tensor_scalar(out=P2[:, 1, 0], in0=tS[:], scalar1=2.0,
                                  scalar2=None, op0=op.mult)
                gp.tensor_scalar(out=P2[:, 1, 1], in0=tSS[:], scalar1=9.0,
                                 scalar2=None, op0=op.mult)

                # x rotations of the (2S, 9SS) planes
                nc.sync.dma_start(out=P2[:, 0][1:128], in_=P2[:, 1][0:127])
                nc.sync.dma_start(out=P2[:, 0][0:1], in_=P2[:, 1][127:128])
                nc.sync.dma_start(out=P2[:, 2][0:127], in_=P2[:, 1][1:128])
                nc.sync.dma_start(out=P2[:, 2][127:128], in_=P2[:, 1][0:1])

                SaO = tS[:, 1:1 + H, 1:1 + YS]
                SSaO = tSS[:, 1:1 + H, 1:1 + YS]
                # var8 = 8*sigma^2 = SSa - Sa^2/9
                dve.scalar_tensor_tensor(out=v1[:], in0=SaO,
                                         scalar=-1.0 / 9.0, in1=SaO,
                                         op0=op.mult, op1=op.mult)
                gp.tensor_tensor(out=v1[:], in0=v1[:], in1=SSaO, op=op.add)
                # zero guard + negsc = -4/(ks^2 * var8), 0 where var8 == 0
                dve.tensor_scalar(out=v2[:], in0=v1[:], scalar1=0.0,
                                  scalar2=None, op0=op.is_equal)
                dve.tensor_tensor(out=v2[:], in0=v2[:], in1=v1[:], op=op.add)
                dve.reciprocal(out=v2[:], in_=v2[:])
                dve.tensor_scalar(out=v1[:], in0=v1[:], scalar1=0.0,
                                  scalar2=None, op0=op.not_equal)
                dve.tensor_tensor(out=v1[:], in0=v1[:], in1=v2[:], op=op.mult)
                # 1/9 absorbs the x9 scaling of the assembly terms
                dve.tensor_scalar(out=negsc[:], in0=v1[:],
                                  scalar1=-4.0 / (9.0 * ks_value * ks_value),
                                  scalar2=None, op0=op.mult)

            # ---------------- dots + select + softmax ----------------
            with tc.tile_pool(name="dotp", bufs=1) as dp:
                for zc in range(0, H, ZC):
                    mark("dots_chunk")
                    # x-rotated Vk slabs covering z rows [zc, zc+34)
                    wrm = dp.tile([128, 8, ZC + 2, YE], f16, tag="wrm")
                    wrp = dp.tile([128, 8, ZC + 2, YE], f16, tag="wrp")
                    src = Vk[:, :, zc:zc + ZC + 2, :]
                    nc.sync.dma_start(out=wrm[1:128], in_=src[0:127])
                    nc.sync.dma_start(out=wrm[0:1], in_=src[127:128])
                    nc.sync.dma_start(out=wrp[0:127], in_=src[1:128])
                    nc.sync.dma_start(out=wrp[127:128], in_=src[0:1])

                    n2fs = [dp.tile([128, 33, 17], f32, name=f"n2{i}",
                                    tag=f"n2{i}") for i in range(3)]
                    n2rs = [dp.tile([128, 33, 17], f32, name=f"nr{i}",
                                    tag=f"nr{i}") for i in range(2)]
                    prods = [dp.tile([128, 8, 33, 17], f16, name=f"pr{i}",
                                     tag=f"pr{i}") for i in range(2)]
                    pt4s = [dp.tile([128, 4, 33, 17], f16, name=f"p4{i}",
                                    tag=f"p4{i}") for i in range(2)]
                    pt2s = [dp.tile([128, 2, 33, 17], f16, name=f"p2{i}",
                                    tag=f"p2{i}") for i in range(2)]
                    dotps = [dp.tile([128, 33, 17], f16, name=f"dp{i}",
                                     tag=f"dp{i}") for i in range(2)]
                    mts = [dp.tile([128, 33, 17], f32, name=f"mt{i}",
                                   tag=f"mt{i}") for i in range(6)]
                    lgs = [dp.tile([128, ZC, YS], f32, name=f"lg{i}",
                                   tag=f"lg{i}") for i in range(2)]
                    ests = [dp.tile([128, ZC, YS], f16, name=f"es{i}",
                                    tag=f"es{i}") for i in range(2)]
                    msk = [dp.tile([128, 8, ZC, YS], u16, name=f"mk{i}",
                                   tag=f"mk{i}") for i in range(2)]
                    acc = dp.tile([128, 8, ZC, YS], f16, tag="acc")

                    scv = negsc[:, zc:zc + ZC, :]
                    icv8 = icod[:, :, zc:zc + ZC, :]

                    def aeng(site):
                        return dve if site in ASM_DVE_SITES else gp

                    def teng(lvl):
                        return gp if lvl in TREE_POOL_LVL else dve

                    for di in POS13:
                        oz, oy, ox = OFFS[di]
                        ezn = 33 if oz else 32
                        eyn = 17 if oy else 16
                        z0 = zc - (1 if oz > 0 else 0)   # global z of ext[0]
                        y0 = -(1 if oy > 0 else 0)       # global y of ext[0]

                        # A-side (voxel p) views over the extended domain
                        XA = X3[:, 1, 1 + z0:1 + z0 + ezn,
                                2 + y0:2 + y0 + eyn]
                        XB = X3[:, ox + 1, 1 + z0 + oz:1 + z0 + oz + ezn,
                                2 + y0 + oy:2 + y0 + oy + eyn]
                        SaV = P2[:, 1, 0, 1 + z0:1 + z0 + ezn,
                                 1 + y0:1 + y0 + eyn]
                        SbV = P2[:, ox + 1, 0,
                                 1 + z0 + oz:1 + z0 + oz + ezn,
                                 1 + y0 + oy:1 + y0 + oy + eyn]
                        SSaV = P2[:, 1, 1, 1 + z0:1 + z0 + ezn,
                                  1 + y0:1 + y0 + eyn]
                        SSbV = P2[:, ox + 1, 1,
                                  1 + z0 + oz:1 + z0 + oz + ezn,
                                  1 + y0 + oy:1 + y0 + oy + eyn]
                        VkA = Vk[:, :, 1 + z0:1 + z0 + ezn,
                                 1 + y0:1 + y0 + eyn]
                        if ox == 0:
                            VkB = Vk[:, :, 1 + z0 + oz:1 + z0 + oz + ezn,
                                     1 + y0 + oy:1 + y0 + oy + eyn]
                        else:
                            wrt = wrm if ox == -1 else wrp
                            # wr z-index = Vk z-index - zc
                            VkB = wrt[:, :, 1 + z0 + oz - zc:
                                      1 + z0 + oz - zc + ezn,
                                      1 + y0 + oy:1 + y0 + oy + eyn]

                        w4 = [mts[(4 * di + j) % 6] for j in range(4)]
                        dfv = w4[0][:, :ezn, :eyn]
                        aeng(0).tensor_tensor(out=dfv, in0=XB, in1=XA,
                                              op=op.subtract)
                        m1 = w4[1][:, :ezn, :eyn]
                        aeng(1).tensor_tensor(out=m1, in0=SbV, in1=SaV,
                                              op=op.subtract)
                        m2 = w4[2][:, :ezn, :eyn]
                        aeng(2).tensor_tensor(out=m2, in0=dfv, in1=m1,
                                              op=op.add)
                        m3 = w4[1][:, :ezn, :eyn]
                        aeng(3).tensor_tensor(out=m3, in0=dfv, in1=m2,
                                              op=op.mult)
                        # 8-wide centered dot over the extended domain
                        prod = prods[di % 2][:, :, :ezn, :eyn]
                        pt4 = pt4s[di % 2][:, :, :ezn, :eyn]
                        pt2 = pt2s[di % 2][:, :, :ezn, :eyn]
                        dotp = dotps[di % 2][:, :ezn, :eyn]
                        dve.tensor_tensor(out=prod, in0=VkA, in1=VkB,
                                          op=op.mult)
                        teng(0).tensor_tensor(out=pt4, in0=prod[:, 0:4],
                                              in1=prod[:, 4:8], op=op.add)
                        teng(1).tensor_tensor(out=pt2, in0=pt4[:, 0:2],
                                              in1=pt4[:, 2:4], op=op.add)
                        teng(2).tensor_tensor(out=dotp, in0=pt2[:, 0],
                                              in1=pt2[:, 1], op=op.add)
                        # 9*(SSb + SSa)
                        m4 = w4[3][:, :ezn, :eyn]
                        aeng(4).tensor_tensor(out=m4, in0=SSbV, in1=SSaV,
                                              op=op.add)
                        m5 = w4[0][:, :ezn, :eyn]
                        dve.scalar_tensor_tensor(out=m5, in0=dotp,
                                                 scalar=-18.0, in1=m4,
                                                 op0=op.mult, op1=op.add)
                        n2f = n2fs[di % 3]
                        n2fv = n2f[:, :ezn, :eyn]
                        aeng(5).tensor_tensor(out=n2fv, in0=m5, in1=m3,
                                              op=op.add)
                        if ox != 0:
                            n2r = n2rs[di % 2]
                            if ox == 1:
                                nc.sync.dma_start(out=n2r[1:128],
                                                  in_=n2f[0:127])
                                nc.sync.dma_start(out=n2r[0:1],
                                                  in_=n2f[127:128])
                            else:
                                nc.sync.dma_start(out=n2r[0:127],
                                                  in_=n2f[1:128])
                                nc.sync.dma_start(out=n2r[127:128],
                                                  in_=n2f[0:1])

                        # two directions: +d reads n2f at p, -d reads the
                        # (x-rotated) field at p-d
                        dplus = di
                        dminus = 26 - di
                        for si, (dd, srct, zo, yo) in enumerate((
                                (dplus, n2f, zc - z0, -y0),
                                (dminus, n2r if ox != 0 else n2f,
                                 zc - z0 - oz, -y0 - oy))):
                            bi = si  # ping-pong within the pair
                            vv = srct[:, zo:zo + ZC, yo:yo + YS]
                            lg = lgs[bi]
                            me = gp if MULT_POOL else dve
                            me.tensor_tensor(out=lg[:], in0=vv, in1=scv,
                                             op=op.mult)
                            ee = ests[bi]
                            act.activation(out=ee[:], in_=lg[:], func=AF.Exp)
                            eb = ee[:].rearrange("p (a z) y -> p a z y", a=1)
                            ebb = eb.broadcast_to([128, 8, ZC, YS])
                            mm = msk[bi]
                            code = float(CODE_OF_D[dd])
                            dve.tensor_scalar(out=mm[:], in0=icv8,
                                              scalar1=code, scalar2=None,
                                              op0=op.is_equal)
                            dve.copy_predicated(out=acc[:], mask=mm[:],
                                                data=ebb)

                    mark("softmax")
                    spt4 = pt4s[0][:, :, :ZC, :YS]
                    spt2 = pt2s[0][:, :, :ZC, :YS]
                    sdot = dotps[0][:, :ZC, :YS]
                    dve.tensor_tensor(out=spt4, in0=acc[:, 0:4],
                                      in1=acc[:, 4:8], op=op.add)
                    dve.tensor_tensor(out=spt2, in0=spt4[:, 0:2],
                                      in1=spt4[:, 2:4], op=op.add)
                    dve.tensor_tensor(out=sdot, in0=spt2[:, 0],
                                      in1=spt2[:, 1], op=op.add)
                    den = mts[0][:, :ZC, :YS]
                    # +1.0 is the center's est (exp(0)); also correct for the
                    # sigma==0 uniform case where every est is 1.
                    dve.tensor_scalar(out=den, in0=sdot, scalar1=1.0,
                                      scalar2=None, op0=op.add)
                    rec = mts[1][:, :ZC, :YS]
                    dve.reciprocal(out=rec, in_=den)
                    ob = dp.tile([128, KN, ZC, YS], f16, tag="ob")
                    act.activation(out=ob[:, 0], in_=rec, func=AF.Copy)
                    for r in range(1, KN):
                        eng = gp if r % 3 == 0 else dve
                        eng.tensor_tensor(out=ob[:, r], in0=acc[:, r - 1],
                                          in1=rec, op=op.mult)
                    nc.sync.dma_start(out=outd[:, :, zc:zc + ZC, :],
                                      in_=ob[:])

    mark("end")
    nc.compile()
    return nc


# --------------------------------------------------------------------------
# Host side
# --------------------------------------------------------------------------

_CACHED = {}


def _get_nc(ks_value):
    key = float(ks_value)
    if key not in _CACHED:
        _CACHED[key] = build_bass(key)
    return _CACHED[key]


def _shard_inputs(x):
    """x: [H, M, N] f32 -> list of per-core xin arrays [128, ZE, YI]."""
    maps = []
    zext = np.arange(-1, H + 1) % H
    for c in range(NCORES):
        ys = (np.arange(YS * c - 2, YS * c + YS + 2)) % M
        slab = x[zext][:, ys, :]                       # [66, 20, 128]
        a = np.ascontiguousarray(slab.transpose(2, 0, 1).astype(np.float32))
        maps.append({"xin": a})
    return maps


def kernel(input, ksigma, k, w):
    from concourse.bass_utils import run_bass_kernel_spmd

    x = np.asarray(input, dtype=np.float32)
    assert x.shape == (H, M, N)
    ks = float(np.asarray(ksigma).reshape(-1)[0])
    assert int(k) == KN and int(w) == 3

    nc = _get_nc(ks)
    in_maps = _shard_inputs(x)
    res = run_bass_kernel_spmd(nc, in_maps, core_ids=list(range(NCORES)))
    full = np.empty((H, M, N, KN), dtype=np.float32)
    for c in range(NCORES):
        oc = res.results[c]["out"]          # [128, KN, H, YS] f16
        full[:, YS * c:YS * c + YS] = oc.transpose(2, 3, 0, 1)
    return full.reshape(H * M * N, KN)


# revision 17
# speedup vs baseline: 13.2884x; 11.3251x over previous
"""Trainium2 Bass kernel for nn_BuildK (27-neighborhood kNN softmax weights).

Strategy: shard the y dimension across 8 NeuronCores (spatial parallel, no
cross-core communication). Each core receives a halo-extended input slab;
the two x-rotated frames are built on-device with partition-shift DMAs.

- Sort keys are u32 bit-packs: (bits(|diff|) & ~0x3F) | (2*w | signbit) built
  with DVE bitwise ops; a pruned top-8-of-26 selection network runs f32
  min/max on the packed keys (positive-float order == bit order, and min/max
  select operands bit-exactly, so the 6-bit payload survives). Network ops
  are split between DVE and Pool (Pool min/max runs at default gpsimd
  efficiency, close to DVE speed, and is otherwise idle in this phase).
- Unpack recovers the signed centered neighbor value Vk_r = v_q - c (pure bit
  ops) and the offset code 2*w per rank.
- Pairwise distances exploit symmetry: 9*n2_d(p) = 9*||W(p)-W(p+d)||^2 is
  computed for only the 13 positive offsets d, over a (+1 z/y halo) extended
  domain; the -d direction reads the same field at view (p-d), with a
  partition-rotate DMA supplying the x-shifted copy when ox != 0. The field
  assembly is exact: 9*n2 = dfv*(dfv + 2Sb - 2Sa) + 9*(SSb+SSa) - 18*dot,
  with an 8-wide centered f16 dot (rank 0 is identically 0).
- logits = n2 * negsc with negsc = -1/(9*2 sigma^2 ks^2) per voxel (the
  reference's +1e-6 eps is negligible under softmax and dropped). exp on
  ScalarE; the center's est is identically 1.0.
- Rank routing: per direction, a u16 is_equal mask (4x-mode TS) against the
  rank-stacked code tile, then copy_predicated writes the est plane
  (broadcast over the 8-rank axis) into the per-rank numerators - the masks
  are disjoint and exhaustive, so no accumulation or init is needed.
- Output is written f16 (softmax weights in [0,1]; quantization ~5e-4 rel)
  and upcast on the host.
"""

import sys

sys.path.insert(0, "/opt/trn_rl_repo")

import numpy as np

H, M, N = 64, 128, 128
NCORES = 8
YS = M // NCORES          # 16 owned y rows per core
YE = YS + 2               # 18 = sort region (owned + 1 halo each side)
YI = YS + 4               # 20 = input slab y extent (halo 2)
ZE = H + 2                # 66 = z extent with periodic wrap rows
KN = 9
ZC = 32                   # z chunk (2 chunks)
FS = ZC * YE              # 576 free elems in sort-phase ops

MASK_HI = 0x7FFFFFC0      # clears sign bit + 6 payload bits
MASK_ID = 0x3E            # payload: offset code 2*w (bits 1..5)

# ---- engine-assignment knobs (tuned against TimelineSim) ----
# Real-ISA constraints (probed on neuronxcc): Pool supports TT add/sub/mult,
# TS mult/is_equal, copies; NOT min/max, NOT bitwise/shift, NOT STT, NOT
# copy_predicated. The sort network and routing are therefore DVE-locked.
NET_POOL_A, NET_POOL_B = 0, 5   # network op i -> Pool iff (i % B) < A (0: ISA-illegal)
PREP_TS_POOL = False            # shift/or pack op (bitwise: DVE only)
UNPACK_TS_POOL = False          # shift-left of unpack (bitwise: DVE only)
ASM_DVE_SITES = ()              # which of the 6 assembly TT sites run on DVE
TREE_POOL_LVL = (1, 2)          # which dot-tree levels (0=pt4,1=pt2,2=dotp) on Pool
MULT_POOL = True                # lg = n2 * scv on Pool


# --------------------------------------------------------------------------
# Selection network: top-8-sorted of the 26 non-center candidates (center is
# always rank 0).
# --------------------------------------------------------------------------

_SORT9 = [(0, 3), (1, 7), (2, 5), (4, 8), (0, 7), (2, 4), (3, 8), (5, 6),
          (0, 2), (1, 3), (4, 5), (7, 8), (1, 4), (3, 6), (5, 7), (0, 1),
          (2, 4), (3, 5), (6, 8), (2, 3), (4, 5), (6, 7), (1, 2), (3, 4),
          (5, 6)]


def _oddeven_merge(lo, n, r, out):
    step = r * 2
    if step < n:
        _oddeven_merge(lo, n, step, out)
        _oddeven_merge(lo + r, n, step, out)
        for i in range(lo + r, lo + n - r, step):
            out.append((i, i + r))
    else:
        out.append((lo, lo + r))


def _merge_topk(lenA, lenB, k):
    ces = []
    _oddeven_merge(0, 32, 1, ces)
    inf = [False] * 32
    for w in range(lenA, 16):
        inf[w] = True
    for w in range(16 + lenB, 32):
        inf[w] = True
    label = list(range(32))
    kept = []
    for (i, j) in ces:
        if inf[i] and inf[j]:
            continue
        if inf[j] and not inf[i]:
            continue
        if inf[i] and not inf[j]:
            label[i], label[j] = label[j], label[i]
            inf[i], inf[j] = False, True
            continue
        kept.append((label[i], label[j]))
    needed = set(label[w] for w in range(k))
    keep = []
    for (i, j) in reversed(kept):
        if i in needed or j in needed:
            keep.append((i, j))
            needed.add(i)
            needed.add(j)
    keep.reverse()

    def rm(w):
        return w if w < 16 else w - 16 + lenA

    return [(rm(i), rm(j)) for (i, j) in keep], [rm(label[w]) for w in range(k)]


def build_network():
    cand = [d for d in range(27) if d != 13]
    S8 = [(0, 1), (2, 3), (4, 5), (6, 7), (0, 2), (1, 3), (4, 6), (5, 7),
          (1, 2), (5, 6), (0, 4), (3, 7), (1, 5), (2, 6), (1, 4), (3, 6),
          (2, 4), (3, 5), (3, 4)]
    net = []
    net += [(i, j) for (i, j) in _SORT9]
    net += [(i + 9, j + 9) for (i, j) in _SORT9]
    net += [(i + 18, j + 18) for (i, j) in S8]
    m1, ow1 = _merge_topk(9, 9, 8)
    net += m1
    m2, ow2 = _merge_topk(8, 8, 8)
    remap = {i: ow1[i] for i in range(8)}
    remap.update({8 + i: 18 + i for i in range(8)})
    net += [(remap[i], remap[j]) for (i, j) in m2]
    outw = [remap[w] for w in ow2]

    live = set(outw)
    ops = []
    for (i, j) in reversed(net):
        ni, nj = i in live, j in live
        if not (ni or nj):
            continue
        ops.append((i, j, ni, nj))
        live.add(i)
        live.add(j)
    ops.reverse()
    return ops, outw, cand


NET_OPS, NET_OUTW, CAND = build_network()

OFFS = [(oz, oy, ox) for oz in (-1, 0, 1) for oy in (-1, 0, 1)
        for ox in (-1, 0, 1)]            # reference enumeration; 13 = center
POS13 = [d for d in range(13)]           # positive offsets: OFFS[0..12]

# code of candidate w (payload bits 1..5); CAND[w] = offset index d
CODE_OF_D = {d: 2 * w for w, d in enumerate(CAND)}

NSLOT = 30


# --------------------------------------------------------------------------
# Bass graph
# --------------------------------------------------------------------------

def build_bass(ks_value: float, reps: int = 1, markers=None):
    from concourse import bacc, mybir
    from concourse import tile
    from concourse.alu_op_type import AluOpType as op

    f32 = mybir.dt.float32
    f16 = mybir.dt.float16
    u32 = mybir.dt.uint32
    u16 = mybir.dt.uint16
    AF = mybir.ActivationFunctionType

    nc = bacc.Bacc("TRN2", target_bir_lowering=False, debug=False,
                   num_devices=NCORES)

    def mark(label):
        if markers is not None:
            markers.append((label, nc.next_id()))

    xin = nc.dram_tensor("xin", [128, ZE, YI], f32, kind="ExternalInput").ap()
    outd = nc.dram_tensor("out", [128, KN, H, YS], f16,
                          kind="ExternalOutput").ap()

    dve = nc.vector
    act = nc.scalar
    gp = nc.gpsimd

    import contextlib

    with tile.TileContext(nc) as tc:
      # reps>1 uses a hardware loop: the NEFF holds ONE copy of the body and
      # repeats it on-device, so marginal-rep timing isn't contaminated by
      # NEFF-size-dependent load time.
      with (tc.For_i(0, reps) if reps > 1 else contextlib.nullcontext()):
        with tc.tile_pool(name="pp", bufs=1) as pp:
            X3 = pp.tile([128, 3, ZE, YI], f32, tag="X3")
            nc.sync.dma_start(out=X3[:, 1], in_=xin[:])
            # on-device x-rotated frames (frame r holds x-col p+r-1 at
            # partition p)
            nc.sync.dma_start(out=X3[:, 0][1:128], in_=X3[:, 1][0:127])
            nc.sync.dma_start(out=X3[:, 0][0:1], in_=X3[:, 1][127:128])
            nc.sync.dma_start(out=X3[:, 2][0:127], in_=X3[:, 1][1:128])
            nc.sync.dma_start(out=X3[:, 2][127:128], in_=X3[:, 1][0:1])
            Vk = pp.tile([128, 8, ZE, YE], f16, tag="Vk")
            icod = pp.tile([128, 8, H, YS], f16, tag="icod")
            maskC = pp.tile([128, 1], u32, tag="maskC")
            dve.memset(maskC[:], MASK_HI)
            P2 = pp.tile([128, 3, 2, ZE, YE], f32, tag="P2")
            negsc = pp.tile([128, H, YS], f32, tag="negsc")

            # ---------------- sort phase ----------------
            with tc.tile_pool(name="sortp", bufs=1) as sp:
                kbig = sp.tile([128, NSLOT, FS], f32, tag="kbig")
                VkT = sp.tile([128, 8, ZC, YE], f32, tag="VkT")
                dtmp = [sp.tile([128, FS], f32, name=f"dt{i}", tag=f"dt{i}")
                        for i in range(3)]
                s2tmp = [sp.tile([128, FS], u32, name=f"st{i}", tag=f"st{i}")
                         for i in range(4)]
                ictmp = [sp.tile([128, FS], u32, name=f"ic{i}", tag=f"ic{i}")
                         for i in range(4)]

                for zc in range(0, H, ZC):
                    mark("sort_chunk")
                    cvw = X3[:, 1, 1 + zc:1 + zc + ZC, 1:1 + YE]

                    def vview(d):
                        oz, oy, ox = OFFS[d]
                        return X3[:, ox + 1,
                                  1 + zc + oz:1 + zc + oz + ZC,
                                  1 + oy:1 + oy + YE]

                    free_slots = list(range(NSLOT))
                    wire_slot = {}

                    def k_ap(s):
                        return kbig[:, s, :]

                    for w, d in enumerate(CAND):
                        s = free_slots.pop()
                        wire_slot[w] = s
                        dt = dtmp[w % 3]
                        gp.tensor_tensor(out=dt[:], in0=vview(d), in1=cvw,
                                         op=op.subtract)
                        du = dt[:].bitcast(u32)
                        # key = (du & ~0x3F & ~sign) | (2w | signbit)
                        st = s2tmp[w % 4]
                        dve.tensor_scalar(out=st[:], in0=du, scalar1=31,
                                          scalar2=2 * w,
                                          op0=op.logical_shift_right,
                                          op1=op.bitwise_or)
                        dve.scalar_tensor_tensor(out=k_ap(s).bitcast(u32),
                                                 in0=du, scalar=maskC[:],
                                                 in1=st[:],
                                                 op0=op.bitwise_and,
                                                 op1=op.bitwise_or)

                    for ni_op, (i, j, ni, nj) in enumerate(NET_OPS):
                        si, sj = wire_slot[i], wire_slot[j]
                        new_i = free_slots.pop() if ni else None
                        new_j = free_slots.pop() if nj else None
                        eng = gp if (ni_op % NET_POOL_B) < NET_POOL_A else dve
                        if ni:
                            eng.tensor_tensor(out=k_ap(new_i), in0=k_ap(si),
                                              in1=k_ap(sj), op=op.min)
                        if nj:
                            eng.tensor_tensor(out=k_ap(new_j), in0=k_ap(si),
                                              in1=k_ap(sj), op=op.max)
                        free_slots.append(si)
                        free_slots.append(sj)
                        if ni:
                            wire_slot[i] = new_i
                        else:
                            del wire_slot[i]
                        if nj:
                            wire_slot[j] = new_j
                        else:
                            del wire_slot[j]

                    # unpack ranks 1..8: Vk (signed centered value) + icod.
                    # VkT = key + (key<<31): the payload bits survive in the
                    # low mantissa (rel 2^-18) and round away in the f16
                    # copy; bit31 add == or since key bit31 is 0.
                    for r in range(1, KN):
                        key = k_ap(wire_slot[NET_OUTW[r - 1]]).bitcast(u32)
                        st = s2tmp[r % 4]
                        dve.tensor_scalar(out=st[:], in0=key, scalar1=31,
                                          scalar2=None,
                                          op0=op.logical_shift_left)
                        gp.tensor_tensor(out=VkT[:, r - 1].bitcast(u32),
                                         in0=key, in1=st[:], op=op.add)
                        ic = ictmp[r % 4]
                        dve.tensor_scalar(out=ic[:], in0=key,
                                          scalar1=MASK_ID, scalar2=None,
                                          op0=op.bitwise_and)
                        icv = ic[:].rearrange("p (z y) -> p z y", z=ZC, y=YE)
                        gp.tensor_copy(out=icod[:, r - 1, zc:zc + ZC, :],
                                       in_=icv[:, :, 1:1 + YS])

                    act.activation(out=Vk[:, :, 1 + zc:1 + zc + ZC, :],
                                   in_=VkT[:], func=AF.Copy)

            # z wrap rows of Vk
            nc.sync.dma_start(out=Vk[:, :, 0:1, :], in_=Vk[:, :, H:H + 1, :])
            nc.sync.dma_start(out=Vk[:, :, ZE - 1:ZE, :], in_=Vk[:, :, 1:2, :])

            # X9 = 9*x, in place over X3 (sort no longer needs raw X3)
            act.activation(out=X3[:], in_=X3[:], func=AF.Copy, scale=9.0)

            # ---------------- stats: Sa, SSa, var, scale ----------------
            mark("stats")
            with tc.tile_pool(name="statp", bufs=1) as stp:
                sq = stp.tile([128, 8, ZE, YE], f16, tag="sq")
                t4 = stp.tile([128, 4, ZE, YE], f16, tag="t4")
                t2 = stp.tile([128, 2, ZE, YE], f16, tag="t2")
                tS = stp.tile([128, ZE, YE], f32, tag="tS")
                tSS = stp.tile([128, ZE, YE], f32, tag="tSS")
                v1 = stp.tile([128, H, YS], f32, tag="v1")
                v2 = stp.tile([128, H, YS], f32, tag="v2")

                dve.tensor_tensor(out=t4[:], in0=Vk[:, 0:4], in1=Vk[:, 4:8],
                                  op=op.add)
                dve.tensor_tensor(out=t2[:], in0=t4[:, 0:2], in1=t4[:, 2:4],
                                  op=op.add)
                dve.tensor_tensor(out=tS[:], in0=t2[:, 0], in1=t2[:, 1],
                                  op=op.add)
                act.activation(out=sq[:], in_=Vk[:], func=AF.Square)
                dve.tensor_tensor(out=t4[:], in0=sq[:, 0:4], in1=sq[:, 4:8],
                                  op=op.add)
                dve.tensor_tensor(out=t2[:], in0=t4[:, 0:2], in1=t4[:, 2:4],
                                  op=op.add)
                dve.tensor_tensor(out=tSS[:], in0=t2[:, 0], in1=t2[:, 1],
                                  op=op.add)
                # P2 stores (2*S, 9*SS) so the n2 assembly is pure TT
                dve.tensor_scalar(out=P2[:, 1, 0], in0=tS[:], scalar1=2.0,
                                  scalar2=None, op0=op.mult)
                gp.tensor_scalar(out=P2[:, 1, 1], in0=tSS[:], scalar1=9.0,
                                 scalar2=None, op0=op.mult)

                # x rotations of the (2S, 9SS) planes
                nc.sync.dma_start(out=P2[:, 0][1:128], in_=P2[:, 1][0:127])
                nc.sync.dma_start(out=P2[:, 0][0:1], in_=P2[:, 1][127:128])
                nc.sync.dma_start(out=P2[:, 2][0:127], in_=P2[:, 1][1:128])
                nc.sync.dma_start(out=P2[:, 2][127:128], in_=P2[:, 1][0:1])

                SaO = tS[:, 1:1 + H, 1:1 + YS]
                SSaO = tSS[:, 1:1 + H, 1:1 + YS]
                # var8 = 8*sigma^2 = SSa - Sa^2/9
                dve.scalar_tensor_tensor(out=v1[:], in0=SaO,
                                         scalar=-1.0 / 9.0, in1=SaO,
                                         op0=op.mult, op1=op.mult)
                gp.tensor_tensor(out=v1[:], in0=v1[:], in1=SSaO, op=op.add)
                # zero guard + negsc = -4/(ks^2 * var8), 0 where var8 == 0
                dve.tensor_scalar(out=v2[:], in0=v1[:], scalar1=0.0,
                                  scalar2=None, op0=op.is_equal)
                dve.tensor_tensor(out=v2[:], in0=v2[:], in1=v1[:], op=op.add)
                dve.reciprocal(out=v2[:], in_=v2[:])
                dve.tensor_scalar(out=v1[:], in0=v1[:], scalar1=0.0,
                                  scalar2=None, op0=op.not_equal)
                dve.tensor_tensor(out=v1[:], in0=v1[:], in1=v2[:], op=op.mult)
                # 1/9 absorbs the x9 scaling of the assembly terms
                dve.tensor_scalar(out=negsc[:], in0=v1[:],
                                  scalar1=-4.0 / (9.0 * ks_value * ks_value),
                                  scalar2=None, op0=op.mult)

            # ---------------- dots + select + softmax ----------------
            with tc.tile_pool(name="dotp", bufs=1) as dp:
                for zc in range(0, H, ZC):
                    mark("dots_chunk")
                    # x-rotated Vk slabs covering z rows [zc, zc+34)
                    wrm = dp.tile([128, 8, ZC + 2, YE], f16, tag="wrm")
                    wrp = dp.tile([128, 8, ZC + 2, YE], f16, tag="wrp")
                    src = Vk[:, :, zc:zc + ZC + 2, :]
                    nc.sync.dma_start(out=wrm[1:128], in_=src[0:127])
                    nc.sync.dma_start(out=wrm[0:1], in_=src[127:128])
                    nc.sync.dma_start(out=wrp[0:127], in_=src[1:128])
                    nc.sync.dma_start(out=wrp[127:128], in_=src[0:1])

                    n2fs = [dp.tile([128, 33, 17], f32, name=f"n2{i}",
                                    tag=f"n2{i}") for i in range(3)]
                    n2rs = [dp.tile([128, 33, 17], f32, name=f"nr{i}",
                                    tag=f"nr{i}") for i in range(2)]
                    prods = [dp.tile([128, 8, 33, 17], f16, name=f"pr{i}",
                                     tag=f"pr{i}") for i in range(2)]
                    pt4s = [dp.tile([128, 4, 33, 17], f16, name=f"p4{i}",
                                    tag=f"p4{i}") for i in range(2)]
                    pt2s = [dp.tile([128, 2, 33, 17], f16, name=f"p2{i}",
                                    tag=f"p2{i}") for i in range(2)]
                    dotps = [dp.tile([128, 33, 17], f16, name=f"dp{i}",
                                     tag=f"dp{i}") for i in range(2)]
                    mts = [dp.tile([128, 33, 17], f32, name=f"mt{i}",
                                   tag=f"mt{i}") for i in range(6)]
                    lgs = [dp.tile([128, ZC, YS], f32, name=f"lg{i}",
                                   tag=f"lg{i}") for i in range(4)]
                    ests = [dp.tile([128, ZC, YS], f16, name=f"es{i}",
                                    tag=f"es{i}") for i in range(4)]
                    msk = [dp.tile([128, 8, ZC, YS], u16, name=f"mk{i}",
                                   tag=f"mk{i}") for i in range(2)]
                    acc = dp.tile([128, 8, ZC, YS], f16, tag="acc")

                    scv = negsc[:, zc:zc + ZC, :]
                    icv8 = icod[:, :, zc:zc + ZC, :]

                    def aeng(site):
                        return dve if site in ASM_DVE_SITES else gp

                    def teng(lvl):
                        return gp if lvl in TREE_POOL_LVL else dve

                    def emit_field(di):
                        oz, oy, ox = OFFS[di]
                        ezn = 33 if oz else 32
                        eyn = 17 if oy else 16
                        z0 = zc - (1 if oz > 0 else 0)   # global z of ext[0]
                        y0 = -(1 if oy > 0 else 0)       # global y of ext[0]

                        # A-side (voxel p) views over the extended domain
                        XA = X3[:, 1, 1 + z0:1 + z0 + ezn,
                                2 + y0:2 + y0 + eyn]
                        XB = X3[:, ox + 1, 1 + z0 + oz:1 + z0 + oz + ezn,
                                2 + y0 + oy:2 + y0 + oy + eyn]
                        SaV = P2[:, 1, 0, 1 + z0:1 + z0 + ezn,
                                 1 + y0:1 + y0 + eyn]
                        SbV = P2[:, ox + 1, 0,
                                 1 + z0 + oz:1 + z0 + oz + ezn,
                                 1 + y0 + oy:1 + y0 + oy + eyn]
                        SSaV = P2[:, 1, 1, 1 + z0:1 + z0 + ezn,
                                  1 + y0:1 + y0 + eyn]
                        SSbV = P2[:, ox + 1, 1,
                                  1 + z0 + oz:1 + z0 + oz + ezn,
                                  1 + y0 + oy:1 + y0 + oy + eyn]
                        VkA = Vk[:, :, 1 + z0:1 + z0 + ezn,
                                 1 + y0:1 + y0 + eyn]
                        if ox == 0:
                            VkB = Vk[:, :, 1 + z0 + oz:1 + z0 + oz + ezn,
                                     1 + y0 + oy:1 + y0 + oy + eyn]
                        else:
                            wrt = wrm if ox == -1 else wrp
                            # wr z-index = Vk z-index - zc
                            VkB = wrt[:, :, 1 + z0 + oz - zc:
                                      1 + z0 + oz - zc + ezn,
                                      1 + y0 + oy:1 + y0 + oy + eyn]

                        w4 = [mts[(4 * di + j) % 6] for j in range(4)]
                        dfv = w4[0][:, :ezn, :eyn]
                        aeng(0).tensor_tensor(out=dfv, in0=XB, in1=XA,
                                              op=op.subtract)
                        m1 = w4[1][:, :ezn, :eyn]
                        aeng(1).tensor_tensor(out=m1, in0=SbV, in1=SaV,
                                              op=op.subtract)
                        m2 = w4[2][:, :ezn, :eyn]
                        aeng(2).tensor_tensor(out=m2, in0=dfv, in1=m1,
                                              op=op.add)
                        m3 = w4[1][:, :ezn, :eyn]
                        aeng(3).tensor_tensor(out=m3, in0=dfv, in1=m2,
                                              op=op.mult)
                        # 8-wide centered dot over the extended domain
                        prod = prods[di % 2][:, :, :ezn, :eyn]
                        pt4 = pt4s[di % 2][:, :, :ezn, :eyn]
                        pt2 = pt2s[di % 2][:, :, :ezn, :eyn]
                        dotp = dotps[di % 2][:, :ezn, :eyn]
                        dve.tensor_tensor(out=prod, in0=VkA, in1=VkB,
                                          op=op.mult)
                        teng(0).tensor_tensor(out=pt4, in0=prod[:, 0:4],
                                              in1=prod[:, 4:8], op=op.add)
                        teng(1).tensor_tensor(out=pt2, in0=pt4[:, 0:2],
                                              in1=pt4[:, 2:4], op=op.add)
                        teng(2).tensor_tensor(out=dotp, in0=pt2[:, 0],
                                              in1=pt2[:, 1], op=op.add)
                        # 9*(SSb + SSa)
                        m4 = w4[3][:, :ezn, :eyn]
                        aeng(4).tensor_tensor(out=m4, in0=SSbV, in1=SSaV,
                                              op=op.add)
                        m5 = w4[0][:, :ezn, :eyn]
                        dve.scalar_tensor_tensor(out=m5, in0=dotp,
                                                 scalar=-18.0, in1=m4,
                                                 op0=op.mult, op1=op.add)
                        n2f = n2fs[di % 3]
                        n2fv = n2f[:, :ezn, :eyn]
                        aeng(5).tensor_tensor(out=n2fv, in0=m5, in1=m3,
                                              op=op.add)
                        n2r = None
                        if ox != 0:
                            n2r = n2rs[di % 2]
                            if ox == 1:
                                nc.sync.dma_start(out=n2r[1:128],
                                                  in_=n2f[0:127])
                                nc.sync.dma_start(out=n2r[0:1],
                                                  in_=n2f[127:128])
                            else:
                                nc.sync.dma_start(out=n2r[0:127],
                                                  in_=n2f[1:128])
                                nc.sync.dma_start(out=n2r[127:128],
                                                  in_=n2f[0:1])
                        return (oz, oy, ox, z0, y0, n2f, n2r)

                    def emit_dirs(di, st8):
                        (oz, oy, ox, z0, y0, n2f, n2r) = st8
                        # two directions: +d reads n2f at p, -d reads the
                        # (x-rotated) field at p-d
                        dplus = di
                        dminus = 26 - di
                        for si, (dd, srct, zo, yo) in enumerate((
                                (dplus, n2f, zc - z0, -y0),
                                (dminus, n2r if ox != 0 else n2f,
                                 zc - z0 - oz, -y0 - oy))):
                            bi = (di % 2) * 2 + si
                            vv = srct[:, zo:zo + ZC, yo:yo + YS]
                            lg = lgs[bi]
                            me = gp if MULT_POOL else dve
                            me.tensor_tensor(out=lg[:], in0=vv, in1=scv,
                                             op=op.mult)
                            ee = ests[bi]
                            act.activation(out=ee[:], in_=lg[:], func=AF.Exp)
                            eb = ee[:].rearrange("p (a z) y -> p a z y", a=1)
                            ebb = eb.broadcast_to([128, 8, ZC, YS])
                            mm = msk[si]
                            code = float(CODE_OF_D[dd])
                            dve.tensor_scalar(out=mm[:], in0=icv8,
                                              scalar1=code, scalar2=None,
                                              op0=op.is_equal)
                            dve.copy_predicated(out=acc[:], mask=mm[:],
                                                data=ebb)

                    # software-pipelined by one pair: pair di's directions
                    # (which wait on the Pool->ACT est chain) are emitted
                    # after pair di+1's field assembly so DVE stays fed.
                    pend = None
                    for di in POS13:
                        st8 = emit_field(di)
                        if pend is not None:
                            emit_dirs(pend[0], pend[1])
                        pend = (di, st8)
                    emit_dirs(pend[0], pend[1])

                    mark("softmax")
                    spt4 = pt4s[0][:, :, :ZC, :YS]
                    spt2 = pt2s[0][:, :, :ZC, :YS]
                    sdot = dotps[0][:, :ZC, :YS]
                    dve.tensor_tensor(out=spt4, in0=acc[:, 0:4],
                                      in1=acc[:, 4:8], op=op.add)
                    dve.tensor_tensor(out=spt2, in0=spt4[:, 0:2],
                                      in1=spt4[:, 2:4], op=op.add)
                    dve.tensor_tensor(out=sdot, in0=spt2[:, 0],
                                      in1=spt2[:, 1], op=op.add)
                    den = mts[0][:, :ZC, :YS]
                    # +1.0 is the center's est (exp(0)); also correct for the
                    # sigma==0 uniform case where every est is 1.
                    dve.tensor_scalar(out=den, in0=sdot, scalar1=1.0,
                                      scalar2=None, op0=op.add)
                    rec = mts[1][:, :ZC, :YS]
                    dve.reciprocal(out=rec, in_=den)
                    ob = dp.tile([128, KN, ZC, YS], f16, tag="ob")
                    act.activation(out=ob[:, 0], in_=rec, func=AF.Copy)
                    for r in range(1, KN):
                        eng = gp if r % 3 == 0 else dve
                        eng.tensor_tensor(out=ob[:, r], in0=acc[:, r - 1],
                                          in1=rec, op=op.mult)
                    nc.sync.dma_start(out=outd[:, :, zc:zc + ZC, :],
                                      in_=ob[:])

    mark("end")
    nc.compile()
    return nc


# --------------------------------------------------------------------------
# Host side
# --------------------------------------------------------------------------

_CACHED = {}


def _get_nc(ks_value):
    key = float(ks_value)
    if key not in _CACHED:
        _CACHED[key] = build_bass(key)
    return _CACHED[key]


def _shard_inputs(x):
    """x: [H, M, N] f32 -> list of per-core xin arrays [128, ZE, YI]."""
    maps = []
    zext = np.arange(-1, H + 1) % H
    for c in range(NCORES):
        ys = (np.arange(YS * c - 2, YS * c + YS + 2)) % M
        slab = x[zext][:, ys, :]                       # [66, 20, 128]
        a = np.ascontiguousarray(slab.transpose(2, 0, 1).astype(np.float32))
        maps.append({"xin": a})
    return maps


def kernel(input, ksigma, k, w):
    from concourse.bass_utils import run_bass_kernel_spmd

    x = np.asarray(input, dtype=np.float32)
    assert x.shape == (H, M, N)
    ks = float(np.asarray(ksigma).reshape(-1)[0])
    assert int(k) == KN and int(w) == 3

    nc = _get_nc(ks)
    in_maps = _shard_inputs(x)
    res = run_bass_kernel_spmd(nc, in_maps, core_ids=list(range(NCORES)))
    full = np.empty((H, M, N, KN), dtype=np.float32)
    for c in range(NCORES):
        oc = res.results[c]["out"]          # [128, KN, H, YS] f16
        full[:, YS * c:YS * c + YS] = oc.transpose(2, 3, 0, 1)
    return full.reshape(H * M * N, KN)


# revision 19
# speedup vs baseline: 39.2122x; 2.9509x over previous
"""Trainium2 Bass kernel for nn_BuildK (27-neighborhood kNN softmax weights).

Strategy: shard the y dimension across 8 NeuronCores (spatial parallel, no
cross-core communication). Each core receives a halo-extended input slab;
the two x-rotated frames are built on-device with partition-shift DMAs.

- Sort keys are u32 bit-packs: (bits(|diff|) & ~0x3F) | (2*w | signbit) built
  with DVE bitwise ops; a pruned top-8-of-26 selection network runs f32
  min/max on the packed keys (positive-float order == bit order, and min/max
  select operands bit-exactly, so the 6-bit payload survives). Network ops
  are split between DVE and Pool (Pool min/max runs at default gpsimd
  efficiency, close to DVE speed, and is otherwise idle in this phase).
- Unpack recovers the signed centered neighbor value Vk_r = v_q - c (pure bit
  ops) and the offset code 2*w per rank.
- Pairwise distances exploit symmetry: 9*n2_d(p) = 9*||W(p)-W(p+d)||^2 is
  computed for only the 13 positive offsets d, over a (+1 z/y halo) extended
  domain; the -d direction reads the same field at view (p-d), with a
  partition-rotate DMA supplying the x-shifted copy when ox != 0. The field
  assembly is exact: 9*n2 = dfv*(dfv + 2Sb - 2Sa) + 9*(SSb+SSa) - 18*dot,
  with an 8-wide centered f16 dot (rank 0 is identically 0).
- logits = n2 * negsc with negsc = -1/(9*2 sigma^2 ks^2) per voxel (the
  reference's +1e-6 eps is negligible under softmax and dropped). exp on
  ScalarE; the center's est is identically 1.0.
- Rank routing: per direction, a u16 is_equal mask (4x-mode TS) against the
  rank-stacked code tile, then copy_predicated writes the est plane
  (broadcast over the 8-rank axis) into the per-rank numerators - the masks
  are disjoint and exhaustive, so no accumulation or init is needed.
- Output is written f16 (softmax weights in [0,1]; quantization ~5e-4 rel)
  and upcast on the host.
"""

import sys

sys.path.insert(0, "/opt/trn_rl_repo")

import numpy as np

H, M, N = 64, 128, 128
NCORES = 8
YS = M // NCORES          # 16 owned y rows per core
YE = YS + 2               # 18 = sort region (owned + 1 halo each side)
YI = YS + 4               # 20 = input slab y extent (halo 2)
ZE = H + 2                # 66 = z extent with periodic wrap rows
KN = 9
ZC = 32                   # z chunk (2 chunks)
FS = ZC * YE              # 576 free elems in sort-phase ops

MASK_HI = 0x7FFFFFC0      # clears sign bit + 6 payload bits
MASK_ID = 0x3E            # payload: offset code 2*w (bits 1..5)

# ---- engine-assignment knobs (tuned against TimelineSim) ----
# Real-ISA constraints (probed on neuronxcc): Pool supports TT add/sub/mult,
# TS mult/is_equal, copies; NOT min/max, NOT bitwise/shift, NOT STT, NOT
# copy_predicated. The sort network and routing are therefore DVE-locked.
NET_POOL_A, NET_POOL_B = 0, 5   # network op i -> Pool iff (i % B) < A (0: ISA-illegal)
PREP_TS_POOL = False            # shift/or pack op (bitwise: DVE only)
UNPACK_TS_POOL = False          # shift-left of unpack (bitwise: DVE only)
ASM_DVE_SITES = ()              # which of the 6 assembly TT sites run on DVE
TREE_POOL_LVL = (2,)            # which dot-tree levels (0=pt4,1=pt2,2=dotp) on Pool
MULT_POOL = True                # lg = n2 * scv on Pool


# --------------------------------------------------------------------------
# Selection network: top-8-sorted of the 26 non-center candidates (center is
# always rank 0).
# --------------------------------------------------------------------------

_SORT9 = [(0, 3), (1, 7), (2, 5), (4, 8), (0, 7), (2, 4), (3, 8), (5, 6),
          (0, 2), (1, 3), (4, 5), (7, 8), (1, 4), (3, 6), (5, 7), (0, 1),
          (2, 4), (3, 5), (6, 8), (2, 3), (4, 5), (6, 7), (1, 2), (3, 4),
          (5, 6)]


def _oddeven_merge(lo, n, r, out):
    step = r * 2
    if step < n:
        _oddeven_merge(lo, n, step, out)
        _oddeven_merge(lo + r, n, step, out)
        for i in range(lo + r, lo + n - r, step):
            out.append((i, i + r))
    else:
        out.append((lo, lo + r))


def _merge_topk(lenA, lenB, k):
    ces = []
    _oddeven_merge(0, 32, 1, ces)
    inf = [False] * 32
    for w in range(lenA, 16):
        inf[w] = True
    for w in range(16 + lenB, 32):
        inf[w] = True
    label = list(range(32))
    kept = []
    for (i, j) in ces:
        if inf[i] and inf[j]:
            continue
        if inf[j] and not inf[i]:
            continue
        if inf[i] and not inf[j]:
            label[i], label[j] = label[j], label[i]
            inf[i], inf[j] = False, True
            continue
        kept.append((label[i], label[j]))
    needed = set(label[w] for w in range(k))
    keep = []
    for (i, j) in reversed(kept):
        if i in needed or j in needed:
            keep.append((i, j))
            needed.add(i)
            needed.add(j)
    keep.reverse()

    def rm(w):
        return w if w < 16 else w - 16 + lenA

    return [(rm(i), rm(j)) for (i, j) in keep], [rm(label[w]) for w in range(k)]


def build_network():
    cand = [d for d in range(27) if d != 13]
    S8 = [(0, 1), (2, 3), (4, 5), (6, 7), (0, 2), (1, 3), (4, 6), (5, 7),
          (1, 2), (5, 6), (0, 4), (3, 7), (1, 5), (2, 6), (1, 4), (3, 6),
          (2, 4), (3, 5), (3, 4)]
    net = []
    net += [(i, j) for (i, j) in _SORT9]
    net += [(i + 9, j + 9) for (i, j) in _SORT9]
    net += [(i + 18, j + 18) for (i, j) in S8]
    # top8(A u B) is contained in top8(A) u top8(B): merges only consume
    # each sorted group's top-8 (a group's 9th element can never be in the
    # global top-8)
    m1, ow1 = _merge_topk(8, 8, 8)

    def rm1(w):
        return w if w < 8 else w + 1
    net += [(rm1(i), rm1(j)) for (i, j) in m1]
    ow1 = [rm1(w) for w in ow1]
    m2, ow2 = _merge_topk(8, 8, 8)
    remap = {i: ow1[i] for i in range(8)}
    remap.update({8 + i: 18 + i for i in range(8)})
    net += [(remap[i], remap[j]) for (i, j) in m2]
    outw = [remap[w] for w in ow2]

    live = set(outw)
    ops = []
    for (i, j) in reversed(net):
        ni, nj = i in live, j in live
        if not (ni or nj):
            continue
        ops.append((i, j, ni, nj))
        live.add(i)
        live.add(j)
    ops.reverse()
    return ops, outw, cand


NET_OPS, NET_OUTW, CAND = build_network()

OFFS = [(oz, oy, ox) for oz in (-1, 0, 1) for oy in (-1, 0, 1)
        for ox in (-1, 0, 1)]            # reference enumeration; 13 = center
POS13 = [d for d in range(13)]           # positive offsets: OFFS[0..12]

# code of candidate w (payload bits 1..5); CAND[w] = offset index d
CODE_OF_D = {d: 2 * w for w, d in enumerate(CAND)}

NSLOT = 30


# --------------------------------------------------------------------------
# Bass graph
# --------------------------------------------------------------------------

def build_bass(ks_value: float, reps: int = 1, markers=None):
    from concourse import bacc, mybir
    from concourse import tile
    from concourse.alu_op_type import AluOpType as op

    f32 = mybir.dt.float32
    f16 = mybir.dt.float16
    u32 = mybir.dt.uint32
    u16 = mybir.dt.uint16
    AF = mybir.ActivationFunctionType

    nc = bacc.Bacc("TRN2", target_bir_lowering=False, debug=False,
                   num_devices=NCORES)

    def mark(label):
        if markers is not None:
            markers.append((label, nc.next_id()))

    xin = nc.dram_tensor("xin", [128, ZE, YI], f32, kind="ExternalInput").ap()
    outd = nc.dram_tensor("out", [128, KN, H, YS], f16,
                          kind="ExternalOutput").ap()

    dve = nc.vector
    act = nc.scalar
    gp = nc.gpsimd

    import contextlib

    with tile.TileContext(nc) as tc:
      # reps>1 uses a hardware loop: the NEFF holds ONE copy of the body and
      # repeats it on-device, so marginal-rep timing isn't contaminated by
      # NEFF-size-dependent load time.
      with (tc.For_i(0, reps) if reps > 1 else contextlib.nullcontext()):
        with tc.tile_pool(name="pp", bufs=1) as pp:
            X3 = pp.tile([128, 3, ZE, YI], f32, tag="X3")
            nc.sync.dma_start(out=X3[:, 1], in_=xin[:])
            # on-device x-rotated frames (frame r holds x-col p+r-1 at
            # partition p)
            nc.sync.dma_start(out=X3[:, 0][1:128], in_=X3[:, 1][0:127])
            nc.sync.dma_start(out=X3[:, 0][0:1], in_=X3[:, 1][127:128])
            nc.sync.dma_start(out=X3[:, 2][0:127], in_=X3[:, 1][1:128])
            nc.sync.dma_start(out=X3[:, 2][127:128], in_=X3[:, 1][0:1])
            Vk = pp.tile([128, 8, ZE, YE], f16, tag="Vk")
            icod = pp.tile([128, 8, H, YS], f16, tag="icod")
            maskC = pp.tile([128, 1], u32, tag="maskC")
            dve.memset(maskC[:], MASK_HI)
            P2 = pp.tile([128, 3, 2, ZE, YE], f32, tag="P2")
            negsc = pp.tile([128, H, YS], f32, tag="negsc")

            # ---------------- sort phase ----------------
            with tc.tile_pool(name="sortp", bufs=1) as sp:
                kbig = sp.tile([128, NSLOT, FS], f32, tag="kbig")
                VkT = sp.tile([128, 8, ZC, YE], f32, tag="VkT")
                dtmp = [sp.tile([128, FS], f32, name=f"dt{i}", tag=f"dt{i}")
                        for i in range(3)]
                s2tmp = [sp.tile([128, FS], u32, name=f"st{i}", tag=f"st{i}")
                         for i in range(4)]
                ictmp = [sp.tile([128, FS], u32, name=f"ic{i}", tag=f"ic{i}")
                         for i in range(4)]

                for zc in range(0, H, ZC):
                    mark("sort_chunk")
                    cvw = X3[:, 1, 1 + zc:1 + zc + ZC, 1:1 + YE]

                    def vview(d):
                        oz, oy, ox = OFFS[d]
                        return X3[:, ox + 1,
                                  1 + zc + oz:1 + zc + oz + ZC,
                                  1 + oy:1 + oy + YE]

                    free_slots = list(range(NSLOT))
                    wire_slot = {}

                    def k_ap(s):
                        return kbig[:, s, :]

                    for w, d in enumerate(CAND):
                        s = free_slots.pop()
                        wire_slot[w] = s
                        dt = dtmp[w % 3]
                        gp.tensor_tensor(out=dt[:], in0=vview(d), in1=cvw,
                                         op=op.subtract)
                        du = dt[:].bitcast(u32)
                        # key = (du & ~0x3F & ~sign) | (2w | signbit)
                        st = s2tmp[w % 4]
                        dve.tensor_scalar(out=st[:], in0=du, scalar1=31,
                                          scalar2=2 * w,
                                          op0=op.logical_shift_right,
                                          op1=op.bitwise_or)
                        dve.scalar_tensor_tensor(out=k_ap(s).bitcast(u32),
                                                 in0=du, scalar=maskC[:],
                                                 in1=st[:],
                                                 op0=op.bitwise_and,
                                                 op1=op.bitwise_or)

                    for ni_op, (i, j, ni, nj) in enumerate(NET_OPS):
                        si, sj = wire_slot[i], wire_slot[j]
                        new_i = free_slots.pop() if ni else None
                        new_j = free_slots.pop() if nj else None
                        eng = gp if (ni_op % NET_POOL_B) < NET_POOL_A else dve
                        if ni:
                            eng.tensor_tensor(out=k_ap(new_i), in0=k_ap(si),
                                              in1=k_ap(sj), op=op.min)
                        if nj:
                            eng.tensor_tensor(out=k_ap(new_j), in0=k_ap(si),
                                              in1=k_ap(sj), op=op.max)
                        free_slots.append(si)
                        free_slots.append(sj)
                        if ni:
                            wire_slot[i] = new_i
                        else:
                            del wire_slot[i]
                        if nj:
                            wire_slot[j] = new_j
                        else:
                            del wire_slot[j]

                    # unpack ranks 1..8: Vk (signed centered value) + icod.
                    # VkT = key + (key<<31): the payload bits survive in the
                    # low mantissa (rel 2^-18) and round away in the f16
                    # copy; bit31 add == or since key bit31 is 0.
                    for r in range(1, KN):
                        key = k_ap(wire_slot[NET_OUTW[r - 1]]).bitcast(u32)
                        st = s2tmp[r % 4]
                        dve.tensor_scalar(out=st[:], in0=key, scalar1=31,
                                          scalar2=None,
                                          op0=op.logical_shift_left)
                        gp.tensor_tensor(out=VkT[:, r - 1].bitcast(u32),
                                         in0=key, in1=st[:], op=op.add)
                        ic = ictmp[r % 4]
                        dve.tensor_scalar(out=ic[:], in0=key,
                                          scalar1=MASK_ID, scalar2=None,
                                          op0=op.bitwise_and)
                        icv = ic[:].rearrange("p (z y) -> p z y", z=ZC, y=YE)
                        gp.tensor_copy(out=icod[:, r - 1, zc:zc + ZC, :],
                                       in_=icv[:, :, 1:1 + YS])

                    act.activation(out=Vk[:, :, 1 + zc:1 + zc + ZC, :],
                                   in_=VkT[:], func=AF.Copy)

            # z wrap rows of Vk
            nc.sync.dma_start(out=Vk[:, :, 0:1, :], in_=Vk[:, :, H:H + 1, :])
            nc.sync.dma_start(out=Vk[:, :, ZE - 1:ZE, :], in_=Vk[:, :, 1:2, :])

            # X9 = 9*x, in place over X3 (sort no longer needs raw X3)
            act.activation(out=X3[:], in_=X3[:], func=AF.Copy, scale=9.0)

            # ---------------- stats: Sa, SSa, var, scale ----------------
            mark("stats")
            with tc.tile_pool(name="statp", bufs=1) as stp:
                sq = stp.tile([128, 8, ZE, YE], f16, tag="sq")
                t4 = stp.tile([128, 4, ZE, YE], f16, tag="t4")
                t2 = stp.tile([128, 2, ZE, YE], f16, tag="t2")
                tS = stp.tile([128, ZE, YE], f32, tag="tS")
                tSS = stp.tile([128, ZE, YE], f32, tag="tSS")
                v1 = stp.tile([128, H, YS], f32, tag="v1")
                v2 = stp.tile([128, H, YS], f32, tag="v2")

                dve.tensor_tensor(out=t4[:], in0=Vk[:, 0:4], in1=Vk[:, 4:8],
                                  op=op.add)
                dve.tensor_tensor(out=t2[:], in0=t4[:, 0:2], in1=t4[:, 2:4],
                                  op=op.add)
                dve.tensor_tensor(out=tS[:], in0=t2[:, 0], in1=t2[:, 1],
                                  op=op.add)
                act.activation(out=sq[:], in_=Vk[:], func=AF.Square)
                dve.tensor_tensor(out=t4[:], in0=sq[:, 0:4], in1=sq[:, 4:8],
                                  op=op.add)
                dve.tensor_tensor(out=t2[:], in0=t4[:, 0:2], in1=t4[:, 2:4],
                                  op=op.add)
                dve.tensor_tensor(out=tSS[:], in0=t2[:, 0], in1=t2[:, 1],
                                  op=op.add)
                # P2 stores (2*S, 9*SS) so the n2 assembly is pure TT
                dve.tensor_scalar(out=P2[:, 1, 0], in0=tS[:], scalar1=2.0,
                                  scalar2=None, op0=op.mult)
                gp.tensor_scalar(out=P2[:, 1, 1], in0=tSS[:], scalar1=9.0,
                                 scalar2=None, op0=op.mult)

                # x rotations of the (2S, 9SS) planes
                nc.sync.dma_start(out=P2[:, 0][1:128], in_=P2[:, 1][0:127])
                nc.sync.dma_start(out=P2[:, 0][0:1], in_=P2[:, 1][127:128])
                nc.sync.dma_start(out=P2[:, 2][0:127], in_=P2[:, 1][1:128])
                nc.sync.dma_start(out=P2[:, 2][127:128], in_=P2[:, 1][0:1])

                SaO = tS[:, 1:1 + H, 1:1 + YS]
                SSaO = tSS[:, 1:1 + H, 1:1 + YS]
                # var8 = 8*sigma^2 = SSa - Sa^2/9
                dve.scalar_tensor_tensor(out=v1[:], in0=SaO,
                                         scalar=-1.0 / 9.0, in1=SaO,
                                         op0=op.mult, op1=op.mult)
                gp.tensor_tensor(out=v1[:], in0=v1[:], in1=SSaO, op=op.add)
                # zero guard + negsc = -4/(ks^2 * var8), 0 where var8 == 0
                dve.tensor_scalar(out=v2[:], in0=v1[:], scalar1=0.0,
                                  scalar2=None, op0=op.is_equal)
                dve.tensor_tensor(out=v2[:], in0=v2[:], in1=v1[:], op=op.add)
                dve.reciprocal(out=v2[:], in_=v2[:])
                dve.tensor_scalar(out=v1[:], in0=v1[:], scalar1=0.0,
                                  scalar2=None, op0=op.not_equal)
                dve.tensor_tensor(out=v1[:], in0=v1[:], in1=v2[:], op=op.mult)
                # 1/9 absorbs the x9 scaling of the assembly terms
                dve.tensor_scalar(out=negsc[:], in0=v1[:],
                                  scalar1=-4.0 / (9.0 * ks_value * ks_value),
                                  scalar2=None, op0=op.mult)

            # ---------------- dots + select + softmax ----------------
            with tc.tile_pool(name="dotp", bufs=1) as dp:
                for zc in range(0, H, ZC):
                    mark("dots_chunk")
                    # x-rotated Vk slabs covering z rows [zc, zc+34)
                    wrm = dp.tile([128, 8, ZC + 2, YE], f16, tag="wrm")
                    wrp = dp.tile([128, 8, ZC + 2, YE], f16, tag="wrp")
                    src = Vk[:, :, zc:zc + ZC + 2, :]
                    nc.sync.dma_start(out=wrm[1:128], in_=src[0:127])
                    nc.sync.dma_start(out=wrm[0:1], in_=src[127:128])
                    nc.sync.dma_start(out=wrp[0:127], in_=src[1:128])
                    nc.sync.dma_start(out=wrp[127:128], in_=src[0:1])

                    n2fs = [dp.tile([128, 33, 17], f32, name=f"n2{i}",
                                    tag=f"n2{i}") for i in range(3)]
                    n2rs = [dp.tile([128, 33, 17], f32, name=f"nr{i}",
                                    tag=f"nr{i}") for i in range(2)]
                    prods = [dp.tile([128, 8, 33, 17], f16, name=f"pr{i}",
                                     tag=f"pr{i}") for i in range(2)]
                    pt4s = [dp.tile([128, 4, 33, 17], f16, name=f"p4{i}",
                                    tag=f"p4{i}") for i in range(2)]
                    pt2s = [dp.tile([128, 2, 33, 17], f16, name=f"p2{i}",
                                    tag=f"p2{i}") for i in range(2)]
                    dotps = [dp.tile([128, 33, 17], f16, name=f"dp{i}",
                                     tag=f"dp{i}") for i in range(2)]
                    mts = [dp.tile([128, 33, 17], f32, name=f"mt{i}",
                                   tag=f"mt{i}") for i in range(6)]
                    lgs = [dp.tile([128, ZC, YS], f32, name=f"lg{i}",
                                   tag=f"lg{i}") for i in range(4)]
                    ests = [dp.tile([128, ZC, YS], f16, name=f"es{i}",
                                    tag=f"es{i}") for i in range(4)]
                    msk = [dp.tile([128, 8, ZC, YS], u16, name=f"mk{i}",
                                   tag=f"mk{i}") for i in range(2)]
                    acc = dp.tile([128, 8, ZC, YS], f16, tag="acc")

                    scv = negsc[:, zc:zc + ZC, :]
                    icv8 = icod[:, :, zc:zc + ZC, :]

                    def aeng(site):
                        return dve if site in ASM_DVE_SITES else gp

                    def teng(lvl):
                        return gp if lvl in TREE_POOL_LVL else dve

                    def emit_field(di):
                        oz, oy, ox = OFFS[di]
                        ezn = 33 if oz else 32
                        eyn = 17 if oy else 16
                        z0 = zc - (1 if oz > 0 else 0)   # global z of ext[0]
                        y0 = -(1 if oy > 0 else 0)       # global y of ext[0]

                        # A-side (voxel p) views over the extended domain
                        XA = X3[:, 1, 1 + z0:1 + z0 + ezn,
                                2 + y0:2 + y0 + eyn]
                        XB = X3[:, ox + 1, 1 + z0 + oz:1 + z0 + oz + ezn,
                                2 + y0 + oy:2 + y0 + oy + eyn]
                        SaV = P2[:, 1, 0, 1 + z0:1 + z0 + ezn,
                                 1 + y0:1 + y0 + eyn]
                        SbV = P2[:, ox + 1, 0,
                                 1 + z0 + oz:1 + z0 + oz + ezn,
                                 1 + y0 + oy:1 + y0 + oy + eyn]
                        SSaV = P2[:, 1, 1, 1 + z0:1 + z0 + ezn,
                                  1 + y0:1 + y0 + eyn]
                        SSbV = P2[:, ox + 1, 1,
                                  1 + z0 + oz:1 + z0 + oz + ezn,
                                  1 + y0 + oy:1 + y0 + oy + eyn]
                        VkA = Vk[:, :, 1 + z0:1 + z0 + ezn,
                                 1 + y0:1 + y0 + eyn]
                        if ox == 0:
                            VkB = Vk[:, :, 1 + z0 + oz:1 + z0 + oz + ezn,
                                     1 + y0 + oy:1 + y0 + oy + eyn]
                        else:
                            wrt = wrm if ox == -1 else wrp
                            # wr z-index = Vk z-index - zc
                            VkB = wrt[:, :, 1 + z0 + oz - zc:
                                      1 + z0 + oz - zc + ezn,
                                      1 + y0 + oy:1 + y0 + oy + eyn]

                        w4 = [mts[(4 * di + j) % 6] for j in range(4)]
                        dfv = w4[0][:, :ezn, :eyn]
                        aeng(0).tensor_tensor(out=dfv, in0=XB, in1=XA,
                                              op=op.subtract)
                        m1 = w4[1][:, :ezn, :eyn]
                        aeng(1).tensor_tensor(out=m1, in0=SbV, in1=SaV,
                                              op=op.subtract)
                        m2 = w4[2][:, :ezn, :eyn]
                        aeng(2).tensor_tensor(out=m2, in0=dfv, in1=m1,
                                              op=op.add)
                        m3 = w4[1][:, :ezn, :eyn]
                        aeng(3).tensor_tensor(out=m3, in0=dfv, in1=m2,
                                              op=op.mult)
                        # 8-wide centered dot over the extended domain
                        prod = prods[di % 2][:, :, :ezn, :eyn]
                        pt4 = pt4s[di % 2][:, :, :ezn, :eyn]
                        pt2 = pt2s[di % 2][:, :, :ezn, :eyn]
                        dotp = dotps[di % 2][:, :ezn, :eyn]
                        dve.tensor_tensor(out=prod, in0=VkA, in1=VkB,
                                          op=op.mult)
                        teng(0).tensor_tensor(out=pt4, in0=prod[:, 0:4],
                                              in1=prod[:, 4:8], op=op.add)
                        teng(1).tensor_tensor(out=pt2, in0=pt4[:, 0:2],
                                              in1=pt4[:, 2:4], op=op.add)
                        teng(2).tensor_tensor(out=dotp, in0=pt2[:, 0],
                                              in1=pt2[:, 1], op=op.add)
                        # 9*(SSb + SSa)
                        m4 = w4[3][:, :ezn, :eyn]
                        aeng(4).tensor_tensor(out=m4, in0=SSbV, in1=SSaV,
                                              op=op.add)
                        m5 = w4[0][:, :ezn, :eyn]
                        dve.scalar_tensor_tensor(out=m5, in0=dotp,
                                                 scalar=-18.0, in1=m4,
                                                 op0=op.mult, op1=op.add)
                        n2f = n2fs[di % 3]
                        n2fv = n2f[:, :ezn, :eyn]
                        aeng(5).tensor_tensor(out=n2fv, in0=m5, in1=m3,
                                              op=op.add)
                        n2r = None
                        if ox != 0:
                            n2r = n2rs[di % 2]
                            if ox == 1:
                                nc.sync.dma_start(out=n2r[1:128],
                                                  in_=n2f[0:127])
                                nc.sync.dma_start(out=n2r[0:1],
                                                  in_=n2f[127:128])
                            else:
                                nc.sync.dma_start(out=n2r[0:127],
                                                  in_=n2f[1:128])
                                nc.sync.dma_start(out=n2r[127:128],
                                                  in_=n2f[0:1])
                        return (oz, oy, ox, z0, y0, n2f, n2r)

                    def emit_dirs(di, st8):
                        (oz, oy, ox, z0, y0, n2f, n2r) = st8
                        # two directions: +d reads n2f at p, -d reads the
                        # (x-rotated) field at p-d
                        dplus = di
                        dminus = 26 - di
                        for si, (dd, srct, zo, yo) in enumerate((
                                (dplus, n2f, zc - z0, -y0),
                                (dminus, n2r if ox != 0 else n2f,
                                 zc - z0 - oz, -y0 - oy))):
                            bi = (di % 2) * 2 + si
                            vv = srct[:, zo:zo + ZC, yo:yo + YS]
                            lg = lgs[bi]
                            me = gp if MULT_POOL else dve
                            me.tensor_tensor(out=lg[:], in0=vv, in1=scv,
                                             op=op.mult)
                            ee = ests[bi]
                            act.activation(out=ee[:], in_=lg[:], func=AF.Exp)
                            eb = ee[:].rearrange("p (a z) y -> p a z y", a=1)
                            ebb = eb.broadcast_to([128, 8, ZC, YS])
                            mm = msk[si]
                            code = float(CODE_OF_D[dd])
                            dve.tensor_scalar(out=mm[:], in0=icv8,
                                              scalar1=code, scalar2=None,
                                              op0=op.is_equal)
                            dve.copy_predicated(out=acc[:], mask=mm[:],
                                                data=ebb)

                    # software-pipelined by one pair: pair di's directions
                    # (which wait on the Pool->ACT est chain) are emitted
                    # after pair di+1's field assembly so DVE stays fed.
                    pend = None
                    for di in POS13:
                        st8 = emit_field(di)
                        if pend is not None:
                            emit_dirs(pend[0], pend[1])
                        pend = (di, st8)
                    emit_dirs(pend[0], pend[1])

                    mark("softmax")
                    spt4 = pt4s[0][:, :, :ZC, :YS]
                    spt2 = pt2s[0][:, :, :ZC, :YS]
                    sdot = dotps[0][:, :ZC, :YS]
                    dve.tensor_tensor(out=spt4, in0=acc[:, 0:4],
                                      in1=acc[:, 4:8], op=op.add)
                    dve.tensor_tensor(out=spt2, in0=spt4[:, 0:2],
                                      in1=spt4[:, 2:4], op=op.add)
                    dve.tensor_tensor(out=sdot, in0=spt2[:, 0],
                                      in1=spt2[:, 1], op=op.add)
                    den = mts[0][:, :ZC, :YS]
                    # +1.0 is the center's est (exp(0)); also correct for the
                    # sigma==0 uniform case where every est is 1.
                    dve.tensor_scalar(out=den, in0=sdot, scalar1=1.0,
                                      scalar2=None, op0=op.add)
                    rec = mts[1][:, :ZC, :YS]
                    dve.reciprocal(out=rec, in_=den)
                    ob = dp.tile([128, KN, ZC, YS], f16, tag="ob")
                    act.activation(out=ob[:, 0], in_=rec, func=AF.Copy)
                    for r in range(1, KN):
                        eng = gp if r % 3 == 0 else dve
                        eng.tensor_tensor(out=ob[:, r], in0=acc[:, r - 1],
                                          in1=rec, op=op.mult)
                    nc.sync.dma_start(out=outd[:, :, zc:zc + ZC, :],
                                      in_=ob[:])

    mark("end")
    nc.compile()
    return nc


# --------------------------------------------------------------------------
# Host side
# --------------------------------------------------------------------------

_CACHED = {}


def _get_nc(ks_value):
    key = float(ks_value)
    if key not in _CACHED:
        _CACHED[key] = build_bass(key)
    return _CACHED[key]


def _shard_inputs(x):
    """x: [H, M, N] f32 -> list of per-core xin arrays [128, ZE, YI]."""
    maps = []
    zext = np.arange(-1, H + 1) % H
    for c in range(NCORES):
        ys = (np.arange(YS * c - 2, YS * c + YS + 2)) % M
        slab = x[zext][:, ys, :]                       # [66, 20, 128]
        a = np.ascontiguousarray(slab.transpose(2, 0, 1).astype(np.float32))
        maps.append({"xin": a})
    return maps


def kernel(input, ksigma, k, w):
    from concourse.bass_utils import run_bass_kernel_spmd

    x = np.asarray(input, dtype=np.float32)
    assert x.shape == (H, M, N)
    ks = float(np.asarray(ksigma).reshape(-1)[0])
    assert int(k) == KN and int(w) == 3

    nc = _get_nc(ks)
    in_maps = _shard_inputs(x)
    res = run_bass_kernel_spmd(nc, in_maps, core_ids=list(range(NCORES)))
    full = np.empty((H, M, N, KN), dtype=np.float32)
    for c in range(NCORES):
        oc = res.results[c]["out"]          # [128, KN, H, YS] f16
        full[:, YS * c:YS * c + YS] = oc.transpose(2, 3, 0, 1)
    return full.reshape(H * M * N, KN)
